# revision 9
# baseline (speedup 1.0000x reference)
"""Chebyshev self-attention Trainium2 kernel (8-core SPMD).

Math restructuring
------------------
reference:  scores = (q @ k.T)/8 + cheb_bias(alphas)[h]  ;  softmax ; @ v

The Chebyshev bias factors exactly as bias_h[i,j] = sum_b G_h[i,b] u_j^b
(degree-5 polynomial in normalized positions); G is computed on the host
from `alphas` and shipped with the inputs, so the bias rides along as extra
contraction rows of the QK matmul.  No [S,S] bias tensor is materialized.

Precision / DoubleRow design (driven by a measured error budget vs the
2e-2 harness gate):

* projections run in bf16 (fp8 hidden-state/weight quantization alone costs
  4e-2 of output error - measured).
* scoresT[j,i] runs as ONE fp8e4 DoubleRow matmul per (j-chunk, i-half)
  with a 210-row contraction folded to [105 partitions, 2 slabs]:
      rows   0..63   q^ . k^          (fp8 of the bf16 projections)
      rows  64..127  q^ . kr          (kr = fp8 residual of k')
      rows 128..191  qr . k^          (qr = fp8 residual of q')
      rows 192..209  3-term bias:  G^.P^ + G^.pr + gr.P^
  The residual rows recover ~bf16 accuracy (the dropped qr.kr / gr.pr
  cross terms are ~0.1%) while the cost model charges a DoubleRow matmul
  half a bf16 one - contraction depth is free.  Residuals cost one DVE
  tensor_sub per projection epilogue; the q^/k^/bias row-blocks are laid
  into the folded aug tiles by free SBUF->SBUF partition-fold DMAs.
* ctx: per 16 j-chunks, the 8 EVEN chunks are "accurate": ACT exp->bf16
  and a bf16 matmul against bf16 v; the 8 ODD chunks pair into 4 fp8e4
  DoubleRow chains against fp8 v PLUS a second DoubleRow chain against the
  fp8 residual of v (fixes v-quantization, measured 1.7e-2 -> ~0.1e-2).
  exp for the odd chunks: half on ACT (exp->fp8), half on DVE as a
  Schraudolph bit-trick (u8(s*256*8*log2e/256 + c) IS the e4m3 bit pattern
  of exp(s)); fp8-pathway probability noise averages over the softmax sum.
* row 64 of each ctx accumulator is the softmax denominator (a constant
  4.0 column in every v tile); the [65, S] numerator/denominator go out in
  bf16 and the host divides + transposes (no PE transposes at all).

Engine balance (cost model): PE ~148us (scores 41 + ctx 102/2 + proj 46),
ACT ~150us (exp), DVE ~140us (Schraudolph exp + residuals + copies).
"""

import numpy as np
from math import comb

B = 4
S = 2048
HIDDEN = 768
HEADS = 12
D = 64
ORDER = 5
NCORES = 8
HG = HEADS // 2          # heads per core (6)
HGDIM = HG * D           # 384 output columns per core
CC = HIDDEN // 128       # contraction chunks (6)
JC = S // 128            # j tiles (16)
VP = 80                  # padded per-head fp8 v row (64 data + 1 denom + pad)
VPB = 66                 # padded per-head bf16 v row
NAUGP = 105              # folded aug partitions (210 rows / 2)

# per-section j-chunk plan: even chunks accurate (ACT exp->bf16, bf16 ctx),
# odd chunks fp8 DoubleRow; engine split for the odd-chunk exp:
A_ACT_JCS = (1, 5, 9, 13)    # odd chunks exp'd on ACT (exp->fp8)
A_DVE_JCS = (3, 7, 11, 15)   # odd chunks exp'd on DVE (Schraudolph u8)

# scale bookkeeping
SQ = 4.0                 # q' = SQ * q
SK = 8.0                 # k' = SK * k   (SQ*SK = 256/8 -> psum = 256*scores)
SB = 16.0                # G' = SB*G, P' = SB*u^b
SV = 4.0                 # v' = SV * v ; denominator column = SV
SCORE_SCALE = 256.0

# blob layout (uint8 offsets, per core)
OFF_HS = 0
N_HS = HIDDEN * S * 2                  # [128, CC, S] bf16
OFF_W = OFF_HS + N_HS
N_W = 3 * HIDDEN * HGDIM * 2           # [3, 128, CC, HGDIM] bf16
OFF_G = OFF_W + N_W
N_G = 3 * 2 * 9 * 2 * S                # [hp, par, 9, 2, S] fp8
OFF_BQK = OFF_G + N_G
N_BQK = 2 * HGDIM * 4                  # [2, 384] f32 raw bytes
NB_NOBIAS = OFF_BQK
NB_BIAS = OFF_BQK + N_BQK

_CACHE = {}


def _cheb_factors(alphas: np.ndarray):
    """alphas [H, 6] -> G [H, 6, S] (i-side, f32), P [6, S] (j-side, f32)."""
    import numpy.polynomial.chebyshev as cheb

    T = np.zeros((ORDER + 1, ORDER + 1))
    for k in range(ORDER + 1):
        e = np.zeros(k + 1)
        e[k] = 1
        T[k, : k + 1] = cheb.cheb2poly(e)[: k + 1]
    c = alphas.astype(np.float64) @ T
    v = np.arange(S, dtype=np.float64) / (S - 1)
    G = np.zeros((HEADS, ORDER + 1, S))
    for h in range(HEADS):
        for b in range(ORDER + 1):
            acc = np.zeros(S)
            for a in range(0, ORDER + 1 - b):
                acc += c[h, a + b] * comb(a + b, a) * ((-v) ** a)
            G[h, b, :] = acc
    P = np.stack([v**b for b in range(ORDER + 1)], 0)
    return G.astype(np.float32), P.astype(np.float32)


def _np_fp8():
    import concourse.mybir as mybir

    return mybir.dt.np(mybir.dt.float8e4)


def _fold35(x6: np.ndarray) -> np.ndarray:
    """[6, S] rows -> folded [3, 2, S] (row 2a+s -> (a, s))."""
    return np.ascontiguousarray(x6.reshape(3, 2, -1))


def _bias_rows(x6: np.ndarray) -> tuple[np.ndarray, np.ndarray]:
    """[6, S] f32 -> (hat [6,S] fp8->f32, residual fp8 bytes) pair."""
    fp8 = _np_fp8()
    hat8 = np.clip(x6, -240, 240).astype(fp8)
    res8 = np.clip(x6 - hat8.astype(np.float32), -240, 240).astype(fp8)
    return hat8, res8


def _build_program(use_qk_bias: bool):
    import concourse.bass as bass
    import concourse.mybir as mybir
    import concourse.tile as tile
    from concourse import bacc

    f32 = mybir.dt.float32
    bf16 = mybir.dt.bfloat16
    fp8 = mybir.dt.float8e4
    u8 = mybir.dt.uint8
    Exp = mybir.ActivationFunctionType.Exp
    Ident = mybir.ActivationFunctionType.Identity
    DR = mybir.MatmulPerfMode.DoubleRow
    MUL = mybir.AluOpType.mult
    ADD = mybir.AluOpType.add

    # Schraudolph constants: e4m3 bits of exp(s) from psum value 256*s
    SCH_C1 = 8.0 * np.log2(np.e) / SCORE_SCALE
    SCH_C2 = 56.0 - 0.5 * 0.458

    nc = bacc.Bacc("TRN2", target_bir_lowering=False, debug=False)

    nb = NB_BIAS if use_qk_bias else NB_NOBIAS
    blob_d = nc.dram_tensor("blob", [nb], u8, kind="ExternalInput")
    out_d = nc.dram_tensor("out", [HG, D + 1, S], f32, kind="ExternalOutput")

    blob = blob_d.ap()
    hs_d = blob[OFF_HS : OFF_HS + N_HS].rearrange(
        "(p c s two) -> p c (s two)", c=CC, s=S, two=2
    )
    w_d = blob[OFF_W : OFF_W + N_W].rearrange(
        "(t p c n two) -> t p c (n two)", t=3, p=128, c=CC, n=HGDIM, two=2
    )
    g_d = blob[OFF_G : OFF_G + N_G].rearrange(
        "(hp par a s i) -> hp par a s i", hp=3, par=2, a=9, s=2, i=S
    )

    # j-side bias rows (fp8): [P^ ; pr ; P^] folded -> [9, 2, S]
    pos = np.arange(S, dtype=np.float64) / (S - 1)
    Pp = (np.stack([(pos**b) for b in range(ORDER + 1)], 0) * SB).astype(np.float32)
    Phat, Pres = _bias_rows(Pp)
    pT9 = np.concatenate(
        [_fold35(Phat.view(np.uint8)), _fold35(Pres.view(np.uint8)), _fold35(Phat.view(np.uint8))], 0
    )
    pT_d = nc.inline_tensor(pT9, name="pT9")

    with tile.TileContext(nc) as tc:
        import contextlib

        with contextlib.ExitStack() as ctx:
            consts = ctx.enter_context(tc.tile_pool(name="consts", bufs=1))
            hsT = consts.tile([128, CC, S], bf16, name="hsT")
            w_sb = [consts.tile([128, CC, HGDIM], bf16, name=f"w{t}") for t in range(3)]
            for cc in range(CC):
                nc.sync.dma_start(out=hsT[:, cc, :].bitcast(u8), in_=hs_d[:, cc, :])
                nc.sync.dma_start(out=w_sb[2][:, cc, :].bitcast(u8), in_=w_d[2, :, cc, :])
            for t in range(2):
                for cc in range(CC):
                    nc.sync.dma_start(
                        out=w_sb[t][:, cc, :].bitcast(u8), in_=w_d[t, :, cc, :]
                    )
            v8 = consts.tile([128, JC // 2, HG, VP], fp8, name="v8")
            vlo = consts.tile([128, JC // 2, HG, VP], fp8, name="vlo")
            vb = consts.tile([128, JC // 2, HG, VPB], bf16, name="vb")
            nc.vector.memset(v8[:, :, :, D : D + 1], SV)
            nc.vector.memset(vlo[:, :, :, D : D + 1], 0.0)
            nc.vector.memset(vb[:, :, :, D : D + 1], SV)
            if use_qk_bias:
                bqk_d = blob[OFF_BQK : OFF_BQK + N_BQK].rearrange(
                    "(t hp p four) -> p t hp four", t=2, hp=3, p=128, four=4
                )
                bqk = consts.tile([128, 2, 3], f32)
                nc.sync.dma_start(out=bqk[:].bitcast(u8), in_=bqk_d)

            aug = ctx.enter_context(tc.tile_pool(name="aug", bufs=2))
            psp = ctx.enter_context(tc.tile_pool(name="psp", bufs=4, space="PSUM"))
            expp = ctx.enter_context(tc.tile_pool(name="expp", bufs=4))
            expb = ctx.enter_context(tc.tile_pool(name="expb", bufs=4))
            stage = ctx.enter_context(tc.tile_pool(name="stage", bufs=8))
            ctxs = ctx.enter_context(tc.tile_pool(name="ctxs", bufs=2))

            # ---- V projection (bf16): v' for all 6 heads; fp8 + residual +
            # bf16 copies per chunk parity ----
            def v_thunks():
                ths = []

                def mk(jc):
                    def f():
                        vp = psp.tile([128, 1024], f32, tag="ps", name="vp")
                        for cc in range(CC):
                            nc.tensor.matmul(
                                vp[:, 0:HGDIM],
                                hsT[:, cc, jc * 128 : (jc + 1) * 128],
                                w_sb[2][:, cc, :],
                                start=(cc == 0),
                                stop=(cc == CC - 1),
                            )
                        vph = vp[:, 0:HGDIM].rearrange("p (h d) -> p h d", d=D)
                        if jc % 2 == 0:
                            nc.vector.tensor_copy(vb[:, jc // 2, :, 0:D], vph)
                        else:
                            a = (jc - 1) // 2
                            nc.vector.tensor_copy(v8[:, a, :, 0:D], vph)
                            nc.vector.tensor_sub(
                                out=vlo[:, a, :, 0:D], in0=vph, in1=v8[:, a, :, 0:D]
                            )
                    return f

                for jc in range(JC):
                    ths.append(mk(jc))
                return ths

            aug_tiles = {}

            def make_aug(hp):
                # folded aug [105, 2, S]: parts 0..31 q^/k^, 32..63 q^/kr,
                # 64..95 qr/k^, 96..104 bias rows
                qe = aug.tile([NAUGP, 2, S], fp8, tag="qaug_e")
                qo = aug.tile([NAUGP, 2, S], fp8, tag="qaug_o")
                ke = aug.tile([NAUGP, 2, S], fp8, tag="kaug_e")
                ko = aug.tile([NAUGP, 2, S], fp8, tag="kaug_o")
                nc.sync.dma_start(out=qe[96:105].bitcast(u8), in_=g_d[hp, 0])
                nc.sync.dma_start(out=qo[96:105].bitcast(u8), in_=g_d[hp, 1])
                nc.sync.dma_start(out=ke[96:105].bitcast(u8), in_=pT_d[:])
                nc.sync.dma_start(out=ko[96:105].bitcast(u8), in_=pT_d[:])
                aug_tiles[hp] = (qe, qo, ke, ko)

            def proj_thunks(hp, t, half):
                """One (tensor, i-half) bf16 projection: 6 cc steps x2 nn into
                a PSUM tile; epilogue: fp8 hat copies (ACT), fp8 residual subs
                (DVE), and 6 partition-fold DMAs into the aug tiles."""
                qe, qo, ke, ko = aug_tiles[hp]
                dst_e, dst_o = (qe, qo) if t == 0 else (ke, ko)
                st = {}

                def step(cc):
                    def f():
                        if cc == 0:
                            st["pp"] = psp.tile([128, 1024], f32, tag="ps", name="pp")
                        pp = st["pp"]
                        for nn in range(2):
                            nc.tensor.matmul(
                                pp[:, nn * 512 : (nn + 1) * 512],
                                w_sb[t][:, cc, hp * 128 : (hp + 1) * 128],
                                hsT[
                                    :,
                                    cc,
                                    half * 1024 + nn * 512 : half * 1024 + (nn + 1) * 512,
                                ],
                                start=(cc == 0),
                                stop=(cc == CC - 1),
                            )
                    return f

                def fin():
                    pp = st["pp"]
                    fs = slice(half * 1024, (half + 1) * 1024)
                    for par, dst in ((0, dst_e), (1, dst_o)):
                        ps = pp[0:D, :] if par == 0 else pp[D:128, :]
                        hat = stage.tile([64, 1024], fp8, tag="stage", name="hat")
                        res = stage.tile([64, 1024], fp8, tag="stage", name="res")
                        if use_qk_bias:
                            nc.scalar.activation(
                                hat[:], ps, Ident, bias=bqk[par * D : (par + 1) * D, t, hp]
                            )
                            nc.vector.memset(res[:], 0.0)
                        else:
                            nc.vector.tensor_copy(hat[:], ps)
                            nc.vector.tensor_sub(out=res[:], in0=ps, in1=hat[:])
                        if t == 0:  # q-side: q^, q^, qr
                            nc.sync.dma_start(out=dst[0:32, :, fs], in_=hat[:])
                            nc.sync.dma_start(out=dst[32:64, :, fs], in_=hat[:])
                            nc.sync.dma_start(out=dst[64:96, :, fs], in_=res[:])
                        else:  # k-side: k^, kr, k^
                            nc.sync.dma_start(out=dst[0:32, :, fs], in_=hat[:])
                            nc.sync.dma_start(out=dst[32:64, :, fs], in_=res[:])
                            nc.sync.dma_start(out=dst[64:96, :, fs], in_=hat[:])

                return [step(cc) for cc in range(CC)] + [fin]

            def tail_thunks(h, ctx_sb):
                def fin():
                    nc.sync.dma_start(out=out_d[h], in_=ctx_sb[:])
                return [fin]

            def attn_section(h, ihalf, ctx_sb, fillq, prev_defer):
                """16-jc loop for one (head, i-half).  Even chunks: ACT
                exp->bf16 + bf16 ctx.  Odd chunks: fp8 pair tiles + DoubleRow
                ctx chains (v8 then vlo).  Tail work deferred to next section."""
                hp, par = divmod(h, 2)
                qa = aug_tiles[hp][par]
                ka = aug_tiles[hp][2 + par]
                st = {"started": False}
                ebs = [None] * (JC // 2)
                eps = [None] * (JC // 4)

                def ctx_mm(lhsT, rhs, last, perf_mode=None):
                    first = not st["started"]
                    st["started"] = True
                    for nn in range(2):
                        rr = (
                            rhs[:, nn * 512 : (nn + 1) * 512]
                            if perf_mode is None
                            else rhs[:, :, nn * 512 : (nn + 1) * 512]
                        )
                        nc.tensor.matmul(
                            st["cp"][0 : D + 1, nn * 512 : (nn + 1) * 512],
                            lhsT,
                            rr,
                            start=first,
                            stop=last,
                            perf_mode=perf_mode,
                        )

                def emit_ctx_b(c, last=False):
                    if "cp" not in st:
                        st["cp"] = psp.tile([128, 1024], f32, tag="ps", name="cp")
                    ctx_mm(vb[:, c // 2, h, 0 : D + 1], ebs[c // 2][:], last)

                def emit_ctx_a(q, last=False):
                    ctx_mm(v8[:, 2 * q : 2 * q + 2, h, 0 : D + 1], eps[q], False, DR)
                    ctx_mm(vlo[:, 2 * q : 2 * q + 2, h, 0 : D + 1], eps[q], last, DR)

                def final_copy():
                    nc.vector.tensor_copy(
                        ctx_sb[:, ihalf * 1024 : (ihalf + 1) * 1024],
                        st["cp"][0 : D + 1, :],
                    )

                for jc in range(JC):
                    if jc < len(prev_defer):
                        prev_defer[jc]()
                    sp = psp.tile([128, 1024], f32, tag="ps", name="sp")
                    for nn in range(2):
                        nc.tensor.matmul(
                            sp[:, nn * 512 : (nn + 1) * 512],
                            ka[:, :, jc * 128 : (jc + 1) * 128],
                            qa[
                                :,
                                :,
                                ihalf * 1024 + nn * 512 : ihalf * 1024 + (nn + 1) * 512,
                            ],
                            start=True,
                            stop=True,
                            perf_mode=DR,
                        )
                    if jc % 2 == 0:
                        ebs[jc // 2] = expb.tile([128, 1024], bf16, tag="expb", name="eb")
                        nc.scalar.activation(
                            ebs[jc // 2][:], sp[:], Exp, scale=1.0 / SCORE_SCALE
                        )
                    else:
                        a = (jc - 1) // 2
                        p, s = divmod(a, 2)
                        if s == 0:
                            eps[p] = expp.tile([128, 2, 1024], fp8, tag="expp", name="ep")
                        if jc in A_DVE_JCS:
                            nc.vector.tensor_scalar(
                                eps[p][:, s, :].bitcast(u8),
                                sp[:],
                                SCH_C1,
                                SCH_C2,
                                MUL,
                                ADD,
                            )
                        else:
                            nc.scalar.activation(
                                eps[p][:, s, :], sp[:], Exp, scale=1.0 / SCORE_SCALE
                            )
                    # lagged ctx: even chunk c at jc=c+2; fp8 pair q at jc=4q+5
                    if jc >= 2 and jc % 2 == 0:
                        emit_ctx_b(jc - 2)
                    elif jc >= 5 and (jc - 1) % 4 == 0:
                        emit_ctx_a((jc - 5) // 4)
                    if jc >= 2 and fillq:
                        fillq.pop(0)()
                        if fillq and jc % 2 == 0:
                            fillq.pop(0)()
                for th in fillq:
                    th()
                return [
                    (lambda: emit_ctx_b(JC - 2)),
                    (lambda: emit_ctx_a(JC // 4 - 1, last=True)),
                    final_copy,
                ]

            # ---- schedule ----
            make_aug(0)
            vths = v_thunks()
            pre = (
                proj_thunks(0, 0, 0)
                + proj_thunks(0, 1, 0)
                + [vths[jc] for jc in (0, 1, 2, 3, 4)]
            )
            for th in pre:
                th()

            ctx_sbs = {}
            defer = []
            pending_tail = []
            # fill order respects emission-order deadlines (a PE consumer must
            # never precede its producer in the PE stream)
            carry = (
                proj_thunks(0, 1, 1)
                + [vths[jc] for jc in (6, 8, 5, 7, 10, 12, 9, 11, 14, 13, 15)]
                + proj_thunks(0, 0, 1)
            )
            for hp in range(3):
                if hp + 1 < 3:
                    make_aug(hp + 1)
                    pj = []
                    for t in range(2):
                        for half in range(2):
                            pj += proj_thunks(hp + 1, t, half)
                else:
                    pj = []
                h_e, h_o = 2 * hp, 2 * hp + 1
                ctx_sbs[h_e] = ctxs.tile([D + 1, S], f32, tag="ctx_sb", name="ctx_sb")
                ctx_sbs[h_o] = ctxs.tile([D + 1, S], f32, tag="ctx_sb", name="ctx_sb")

                nfill = 21
                if hp == 0:
                    s0_fill = carry + pending_tail
                    s1_fill = pj[:nfill]
                    pj = pj[nfill:]
                else:
                    q0 = carry + pending_tail
                    s0_fill, rest = q0[:nfill], q0[nfill:]
                    s1_fill = rest + pj[: nfill - len(rest)]
                    pj = pj[nfill - len(rest) :]
                carry = []

                defer = attn_section(h_e, 0, ctx_sbs[h_e], s0_fill, defer)
                defer = attn_section(h_e, 1, ctx_sbs[h_e], s1_fill, defer)

                te = tail_thunks(h_e, ctx_sbs[h_e])
                s2_fill, pj = (te + pj)[:nfill], (te + pj)[nfill:]
                s3_fill = pj
                defer = attn_section(h_o, 0, ctx_sbs[h_o], s2_fill, defer)
                defer = attn_section(h_o, 1, ctx_sbs[h_o], s3_fill, defer)

                pending_tail = tail_thunks(h_o, ctx_sbs[h_o])

            for th in defer:
                th()
            for th in pending_tail:
                th()

    nc.finalize()
    return nc


def _get_nc(use_qk_bias: bool):
    key = ("prog", use_qk_bias)
    if key not in _CACHE:
        _CACHE[key] = _build_program(use_qk_bias)
    return _CACHE[key]


def _get_runner(use_qk_bias: bool):
    """Cached PJRT runner: one jitted executable + resident zero-output
    buffers; per call only the input blob is uploaded."""
    key = ("runner", use_qk_bias)
    if key in _CACHE:
        return _CACHE[key]

    nc = _get_nc(use_qk_bias)

    import jax
    from jax.sharding import Mesh, PartitionSpec
    from jax.experimental.shard_map import shard_map
    import concourse.mybir as mybir
    from concourse.bass2jax import (
        _bass_exec_p,
        install_neuronx_cc_hook,
        partition_id_tensor,
    )

    install_neuronx_cc_hook()
    partition_name = nc.partition_id_tensor.name if nc.partition_id_tensor else None
    in_names, out_names, out_avals, zero_outs = [], [], [], []
    for alloc in nc.m.functions[0].allocations:
        if not isinstance(alloc, mybir.MemoryLocationSet):
            continue
        name = alloc.memorylocations[0].name
        if alloc.kind == "ExternalInput":
            if name != partition_name:
                in_names.append(name)
        elif alloc.kind == "ExternalOutput":
            out_names.append(name)
            shape = tuple(alloc.tensor_shape)
            dtype = mybir.dt.np(alloc.dtype)
            out_avals.append(jax.core.ShapedArray(shape, dtype))
            zero_outs.append(np.zeros(shape, dtype))
    assert in_names == ["blob"] and out_names == ["out"]
    all_in_names = list(in_names) + list(out_names)
    if partition_name is not None:
        all_in_names.append(partition_name)

    def _body(*args):
        operands = list(args)
        if partition_name is not None:
            operands.append(partition_id_tensor())
        return tuple(
            _bass_exec_p.bind(
                *operands,
                out_avals=tuple(out_avals),
                in_names=tuple(all_in_names),
                out_names=tuple(out_names),
                lowering_input_output_aliases=(),
                sim_require_finite=True,
                sim_require_nnan=True,
                nc=nc,
            )
        )

    devices = jax.devices()[:NCORES]
    mesh = Mesh(np.asarray(devices), ("core",))
    sharded = jax.jit(
        shard_map(
            _body,
            mesh=mesh,
            in_specs=(PartitionSpec("core"),) * 2,
            out_specs=(PartitionSpec("core"),),
            check_rep=False,
        ),
        keep_unused=True,
    )
    dev_zeros = jax.device_put(
        np.zeros(
            (NCORES * zero_outs[0].shape[0], *zero_outs[0].shape[1:]),
            zero_outs[0].dtype,
        )
    )
    out_shape = tuple(out_avals[0].shape)

    def run(blobs: np.ndarray):
        (out,) = sharded(blobs, dev_zeros)
        return np.asarray(out).reshape(NCORES, *out_shape)

    _CACHE[key] = run
    return run


def _make_in_maps(hidden_states, Wq, bq, Wk, bk, Wv, bv, alphas, use_qk_bias):
    """Per-core packed blobs (uint8 1-D), as list of dicts keyed 'blob'."""
    import ml_dtypes

    bf = ml_dtypes.bfloat16
    G, _ = _cheb_factors(alphas)  # [12, 6, S] f32

    hs_b = []
    for b in range(B):
        t = np.ascontiguousarray(hidden_states[b].T)      # [768, S]
        t = t.reshape(CC, 128, S).transpose(1, 0, 2)      # [128, CC, S]
        hs_b.append(np.ascontiguousarray(t).astype(bf).view(np.uint8).ravel())

    w_hg, g_hg, bqk_hg = [], [], []
    for hg in range(2):
        rows = slice(hg * HGDIM, (hg + 1) * HGDIM)
        ws = []
        for W, s in ((Wq, SQ), (Wk, SK), (Wv, SV)):
            wt = np.ascontiguousarray(W[rows, :].T) * s       # [768, 384]
            wt = wt.reshape(CC, 128, HGDIM).transpose(1, 0, 2)
            ws.append(np.ascontiguousarray(wt).astype(bf).view(np.uint8).ravel())
        w_hg.append(np.concatenate(ws))
        gg = np.empty((3, 2, 9, 2, S), dtype=np.uint8)
        for hp in range(3):
            for par in range(2):
                Ghat, Gres = _bias_rows(SB * G[hg * HG + 2 * hp + par])
                gg[hp, par] = np.concatenate(
                    [
                        _fold35(Ghat.view(np.uint8)),
                        _fold35(Ghat.view(np.uint8)),
                        _fold35(Gres.view(np.uint8)),
                    ],
                    0,
                )
        g_hg.append(gg.ravel())
        bqk_hg.append(
            np.ascontiguousarray(
                np.stack([bq[rows] * SQ, bk[rows] * SK], 0), dtype=np.float32
            )
            .ravel()
            .view(np.uint8)
        )

    in_maps = []
    for core in range(NCORES):
        b, hg = divmod(core, 2)
        parts = [hs_b[b], w_hg[hg], g_hg[hg]]
        if use_qk_bias:
            parts.append(bqk_hg[hg])
        in_maps.append({"blob": np.concatenate(parts)})
    return in_maps


def kernel(hidden_states, Wq, bq, Wk, bk, Wv, bv, alphas):
    hidden_states = np.asarray(hidden_states, dtype=np.float32)
    Wq = np.asarray(Wq, dtype=np.float32)
    Wk = np.asarray(Wk, dtype=np.float32)
    Wv = np.asarray(Wv, dtype=np.float32)
    bq = np.asarray(bq, dtype=np.float32)
    bk = np.asarray(bk, dtype=np.float32)
    bv = np.asarray(bv, dtype=np.float32)
    alphas = np.asarray(alphas, dtype=np.float32)

    use_qk_bias = bool(np.any(bq) or np.any(bk))
    run = _get_runner(use_qk_bias)
    in_maps = _make_in_maps(
        hidden_states, Wq, bq, Wk, bk, Wv, bv, alphas, use_qk_bias
    )
    blobs = np.concatenate([m["blob"] for m in in_maps])
    res = run(blobs)  # [8, HG, 65, S] f32

    resf = np.asarray(res, dtype=np.float32)
    num = resf[:, :, 0:D, :]
    den = resf[:, :, D : D + 1, :]
    ctx = num / den
    ctx = ctx.transpose(0, 3, 1, 2).reshape(NCORES, S, HGDIM)

    out = np.empty((B, S, HIDDEN), dtype=np.float32)
    add_bv = bool(np.any(bv))
    for core in range(NCORES):
        b, hg = divmod(core, 2)
        o = ctx[core]
        if add_bv:
            o = o + bv[hg * HGDIM : (hg + 1) * HGDIM][None, :]
        out[b, :, hg * HGDIM : (hg + 1) * HGDIM] = o
    return out


# revision 10
# speedup vs baseline: 1.3649x; 1.3649x over previous
"""Chebyshev self-attention Trainium2 kernel (8-core SPMD).

Math restructuring
------------------
reference:  scores = (q @ k.T)/8 + cheb_bias(alphas)[h]  ;  softmax ; @ v

The Chebyshev bias factors exactly as bias_h[i,j] = sum_b G_h[i,b] u_j^b
(degree-5 polynomial in normalized positions); G is computed on the host
from `alphas` and shipped with the inputs, so the bias rides along as extra
contraction rows of the QK matmul.  No [S,S] bias tensor is materialized.

Precision / DoubleRow design (driven by a measured error budget vs the
2e-2 harness gate):

* projections run in bf16 (fp8 hidden-state/weight quantization alone costs
  4e-2 of output error - measured).
* scoresT[j,i] runs as ONE fp8e4 DoubleRow matmul per (j-chunk, i-half)
  with a 210-row contraction folded to [105 partitions, 2 slabs]:
      rows   0..63   q^ . k^          (fp8 of the bf16 projections)
      rows  64..127  q^ . kr          (kr = fp8 residual of k')
      rows 128..191  qr . k^          (qr = fp8 residual of q')
      rows 192..209  3-term bias:  G^.P^ + G^.pr + gr.P^
  The residual rows recover ~bf16 accuracy (the dropped qr.kr / gr.pr
  cross terms are ~0.1%) while the cost model charges a DoubleRow matmul
  half a bf16 one - contraction depth is free.  Residuals cost one DVE
  tensor_sub per projection epilogue; the q^/k^/bias row-blocks are laid
  into the folded aug tiles by free SBUF->SBUF partition-fold DMAs.
* ctx: per 16 j-chunks, the 8 EVEN chunks are "accurate": ACT exp->bf16
  and a bf16 matmul against bf16 v; the 8 ODD chunks pair into 4 fp8e4
  DoubleRow chains against fp8 v PLUS a second DoubleRow chain against the
  fp8 residual of v (fixes v-quantization, measured 1.7e-2 -> ~0.1e-2).
  exp for the odd chunks: half on ACT (exp->fp8), half on DVE as a
  Schraudolph bit-trick (u8(s*256*8*log2e/256 + c) IS the e4m3 bit pattern
  of exp(s)); fp8-pathway probability noise averages over the softmax sum.
* row 64 of each ctx accumulator is the softmax denominator (a constant
  4.0 column in every v tile); the [65, S] numerator/denominator go out in
  bf16 and the host divides + transposes (no PE transposes at all).

Engine balance (cost model): PE ~148us (scores 41 + ctx 102/2 + proj 46),
ACT ~150us (exp), DVE ~140us (Schraudolph exp + residuals + copies).
"""

import numpy as np
from math import comb

B = 4
S = 2048
HIDDEN = 768
HEADS = 12
D = 64
ORDER = 5
NCORES = 8
HG = HEADS // 2          # heads per core (6)
HGDIM = HG * D           # 384 output columns per core
CC = HIDDEN // 128       # contraction chunks (6)
JC = S // 128            # j tiles (16)
VP = 80                  # padded per-head fp8 v row (64 data + 1 denom + pad)
VPB = 66                 # padded per-head bf16 v row
NAUGP = 105              # folded aug partitions (210 rows / 2)

# per-section j-chunk plan: even chunks accurate (ACT exp->bf16, bf16 ctx),
# odd chunks fp8 DoubleRow; engine split for the odd-chunk exp:
A_ACT_JCS = (1, 5, 9, 13)    # odd chunks exp'd on ACT (exp->fp8)
A_DVE_JCS = (3, 7, 11, 15)   # odd chunks exp'd on DVE (Schraudolph u8)

# scale bookkeeping
SQ = 4.0                 # q' = SQ * q
SK = 8.0                 # k' = SK * k   (SQ*SK = 256/8 -> psum = 256*scores)
SB = 16.0                # G' = SB*G, P' = SB*u^b
SV = 4.0                 # v' = SV * v ; denominator column = SV
SCORE_SCALE = 256.0

# blob layout (uint8 offsets, per core)
OFF_HS = 0
N_HS = HIDDEN * S * 2                  # [128, CC, S] bf16
OFF_W = OFF_HS + N_HS
N_W = 3 * HIDDEN * HGDIM * 2           # [3, 128, CC, HGDIM] bf16
OFF_G = OFF_W + N_W
N_G = 3 * 2 * 9 * 2 * S                # [hp, par, 9, 2, S] fp8
OFF_BQK = OFF_G + N_G
N_BQK = 2 * HGDIM * 4                  # [2, 384] f32 raw bytes
NB_NOBIAS = OFF_BQK
NB_BIAS = OFF_BQK + N_BQK

_CACHE = {}


def _cheb_factors(alphas: np.ndarray):
    """alphas [H, 6] -> G [H, 6, S] (i-side, f32), P [6, S] (j-side, f32)."""
    import numpy.polynomial.chebyshev as cheb

    T = np.zeros((ORDER + 1, ORDER + 1))
    for k in range(ORDER + 1):
        e = np.zeros(k + 1)
        e[k] = 1
        T[k, : k + 1] = cheb.cheb2poly(e)[: k + 1]
    c = alphas.astype(np.float64) @ T
    v = np.arange(S, dtype=np.float64) / (S - 1)
    G = np.zeros((HEADS, ORDER + 1, S))
    for h in range(HEADS):
        for b in range(ORDER + 1):
            acc = np.zeros(S)
            for a in range(0, ORDER + 1 - b):
                acc += c[h, a + b] * comb(a + b, a) * ((-v) ** a)
            G[h, b, :] = acc
    P = np.stack([v**b for b in range(ORDER + 1)], 0)
    return G.astype(np.float32), P.astype(np.float32)


def _np_fp8():
    import concourse.mybir as mybir

    return mybir.dt.np(mybir.dt.float8e4)


def _fold35(x6: np.ndarray) -> np.ndarray:
    """[6, S] rows -> folded [3, 2, S] (row 2a+s -> (a, s))."""
    return np.ascontiguousarray(x6.reshape(3, 2, -1))


def _bias_rows(x6: np.ndarray) -> tuple[np.ndarray, np.ndarray]:
    """[6, S] f32 -> (hat [6,S] fp8->f32, residual fp8 bytes) pair."""
    fp8 = _np_fp8()
    hat8 = np.clip(x6, -240, 240).astype(fp8)
    res8 = np.clip(x6 - hat8.astype(np.float32), -240, 240).astype(fp8)
    return hat8, res8


def _build_program(use_qk_bias: bool):
    import concourse.bass as bass
    import concourse.mybir as mybir
    import concourse.tile as tile
    from concourse import bacc

    f32 = mybir.dt.float32
    bf16 = mybir.dt.bfloat16
    fp8 = mybir.dt.float8e4
    u8 = mybir.dt.uint8
    Exp = mybir.ActivationFunctionType.Exp
    Ident = mybir.ActivationFunctionType.Identity
    DR = mybir.MatmulPerfMode.DoubleRow
    MUL = mybir.AluOpType.mult
    ADD = mybir.AluOpType.add

    # Schraudolph constants: e4m3 bits of exp(s) from psum value 256*s
    SCH_C1 = 8.0 * np.log2(np.e) / SCORE_SCALE
    SCH_C2 = 56.0 - 0.5 * 0.458

    nc = bacc.Bacc("TRN2", target_bir_lowering=False, debug=False)

    nb = NB_BIAS if use_qk_bias else NB_NOBIAS
    blob_d = nc.dram_tensor("blob", [nb], u8, kind="ExternalInput")
    out_d = nc.dram_tensor("out", [HG, D + 1, S], f32, kind="ExternalOutput")

    blob = blob_d.ap()
    hs_d = blob[OFF_HS : OFF_HS + N_HS].rearrange(
        "(p c s two) -> p c (s two)", c=CC, s=S, two=2
    )
    w_d = blob[OFF_W : OFF_W + N_W].rearrange(
        "(t p c n two) -> t p c (n two)", t=3, p=128, c=CC, n=HGDIM, two=2
    )
    g_d = blob[OFF_G : OFF_G + N_G].rearrange(
        "(hp par a s i) -> hp par a s i", hp=3, par=2, a=9, s=2, i=S
    )

    # j-side bias rows (fp8): [P^ ; pr ; P^] folded -> [9, 2, S]
    pos = np.arange(S, dtype=np.float64) / (S - 1)
    Pp = (np.stack([(pos**b) for b in range(ORDER + 1)], 0) * SB).astype(np.float32)
    Phat, Pres = _bias_rows(Pp)
    pT9 = np.concatenate(
        [_fold35(Phat.view(np.uint8)), _fold35(Pres.view(np.uint8)), _fold35(Phat.view(np.uint8))], 0
    )
    pT_d = nc.inline_tensor(pT9, name="pT9")

    with tile.TileContext(nc) as tc:
        import contextlib

        with contextlib.ExitStack() as ctx:
            consts = ctx.enter_context(tc.tile_pool(name="consts", bufs=1))
            hsT = consts.tile([128, CC, S], bf16, name="hsT")
            w_sb = [consts.tile([128, CC, HGDIM], bf16, name=f"w{t}") for t in range(3)]
            nc.sync.dma_start(out=w_sb[0][:].bitcast(u8), in_=w_d[0])
            nc.sync.dma_start(out=hsT[:, 0, :].bitcast(u8), in_=hs_d[:, 0, :])
            nc.sync.dma_start(out=hsT[:, 1, :].bitcast(u8), in_=hs_d[:, 1, :])
            nc.sync.dma_start(out=w_sb[1][:].bitcast(u8), in_=w_d[1])
            nc.sync.dma_start(out=hsT[:, 2, :].bitcast(u8), in_=hs_d[:, 2, :])
            nc.sync.dma_start(out=w_sb[2][:].bitcast(u8), in_=w_d[2])
            for cc in range(3, CC):
                nc.sync.dma_start(out=hsT[:, cc, :].bitcast(u8), in_=hs_d[:, cc, :])
            v8 = consts.tile([128, JC // 2, HG, VP], fp8, name="v8")
            vlo = consts.tile([128, JC // 2, HG, VP], fp8, name="vlo")
            vb = consts.tile([128, JC // 2, HG, VPB], bf16, name="vb")
            nc.vector.memset(v8[:, :, :, D : D + 1], SV)
            nc.vector.memset(vlo[:, :, :, D : D + 1], 0.0)
            nc.vector.memset(vb[:, :, :, D : D + 1], SV)
            if use_qk_bias:
                bqk_d = blob[OFF_BQK : OFF_BQK + N_BQK].rearrange(
                    "(t hp p four) -> p t hp four", t=2, hp=3, p=128, four=4
                )
                bqk = consts.tile([128, 2, 3], f32)
                nc.sync.dma_start(out=bqk[:].bitcast(u8), in_=bqk_d)

            aug = ctx.enter_context(tc.tile_pool(name="aug", bufs=2))
            psp = ctx.enter_context(tc.tile_pool(name="psp", bufs=4, space="PSUM"))
            expp = ctx.enter_context(tc.tile_pool(name="expp", bufs=4))
            expb = ctx.enter_context(tc.tile_pool(name="expb", bufs=4))
            stage = ctx.enter_context(tc.tile_pool(name="stage", bufs=8))
            ctxs = ctx.enter_context(tc.tile_pool(name="ctxs", bufs=2))

            # ---- V projection (bf16): v' for all 6 heads; fp8 + residual +
            # bf16 copies per chunk parity ----
            def v_thunks():
                ths = []

                def mk(jc):
                    def f():
                        vp = psp.tile([128, 1024], f32, tag="ps", name="vp")
                        for cc in range(CC):
                            nc.tensor.matmul(
                                vp[:, 0:HGDIM],
                                hsT[:, cc, jc * 128 : (jc + 1) * 128],
                                w_sb[2][:, cc, :],
                                start=(cc == 0),
                                stop=(cc == CC - 1),
                            )
                        vph = vp[:, 0:HGDIM].rearrange("p (h d) -> p h d", d=D)
                        if jc % 2 == 0:
                            nc.vector.tensor_copy(vb[:, jc // 2, :, 0:D], vph)
                        else:
                            a = (jc - 1) // 2
                            nc.vector.tensor_copy(v8[:, a, :, 0:D], vph)
                            nc.vector.tensor_sub(
                                out=vlo[:, a, :, 0:D], in0=vph, in1=v8[:, a, :, 0:D]
                            )
                    return f

                for jc in range(JC):
                    ths.append(mk(jc))
                return ths

            aug_tiles = {}

            def make_aug(hp):
                # folded aug [105, 2, S]: parts 0..31 q^/k^, 32..63 q^/kr,
                # 64..95 qr/k^, 96..104 bias rows
                qe = aug.tile([NAUGP, 2, S], fp8, tag="qaug_e")
                qo = aug.tile([NAUGP, 2, S], fp8, tag="qaug_o")
                ke = aug.tile([NAUGP, 2, S], fp8, tag="kaug_e")
                ko = aug.tile([NAUGP, 2, S], fp8, tag="kaug_o")
                nc.gpsimd.dma_start(out=qe[96:105].bitcast(u8), in_=g_d[hp, 0])
                nc.gpsimd.dma_start(out=qo[96:105].bitcast(u8), in_=g_d[hp, 1])
                nc.gpsimd.dma_start(out=ke[96:105].bitcast(u8), in_=pT_d[:])
                nc.gpsimd.dma_start(out=ko[96:105].bitcast(u8), in_=pT_d[:])
                aug_tiles[hp] = (qe, qo, ke, ko)

            def proj_thunks(hp, t, half):
                """One (tensor, i-half) bf16 projection: 6 cc steps x2 nn into
                a PSUM tile; epilogue: fp8 hat copies (ACT), fp8 residual subs
                (DVE), and 6 partition-fold DMAs into the aug tiles."""
                qe, qo, ke, ko = aug_tiles[hp]
                dst_e, dst_o = (qe, qo) if t == 0 else (ke, ko)
                st = {}

                def step(cc):
                    def f():
                        if cc == 0:
                            st["pp"] = psp.tile([128, 1024], f32, tag="ps", name="pp")
                        pp = st["pp"]
                        for nn in range(2):
                            nc.tensor.matmul(
                                pp[:, nn * 512 : (nn + 1) * 512],
                                w_sb[t][:, cc, hp * 128 : (hp + 1) * 128],
                                hsT[
                                    :,
                                    cc,
                                    half * 1024 + nn * 512 : half * 1024 + (nn + 1) * 512,
                                ],
                                start=(cc == 0),
                                stop=(cc == CC - 1),
                            )
                    return f

                def fin():
                    pp = st["pp"]
                    fs = slice(half * 1024, (half + 1) * 1024)
                    for par, dst in ((0, dst_e), (1, dst_o)):
                        ps = pp[0:D, :] if par == 0 else pp[D:128, :]
                        hat = stage.tile([64, 1024], fp8, tag="stage", name="hat")
                        res = stage.tile([64, 1024], fp8, tag="stage", name="res")
                        if use_qk_bias:
                            nc.scalar.activation(
                                hat[:], ps, Ident, bias=bqk[par * D : (par + 1) * D, t, hp]
                            )
                            nc.vector.memset(res[:], 0.0)
                        else:
                            nc.vector.tensor_copy(hat[:], ps)
                            nc.vector.tensor_sub(out=res[:], in0=ps, in1=hat[:])
                        if t == 0:  # q-side: q^, q^, qr
                            nc.gpsimd.dma_start(out=dst[0:32, :, fs], in_=hat[:])
                            nc.gpsimd.dma_start(out=dst[32:64, :, fs], in_=hat[:])
                            nc.gpsimd.dma_start(out=dst[64:96, :, fs], in_=res[:])
                        else:  # k-side: k^, kr, k^
                            nc.gpsimd.dma_start(out=dst[0:32, :, fs], in_=hat[:])
                            nc.gpsimd.dma_start(out=dst[32:64, :, fs], in_=res[:])
                            nc.gpsimd.dma_start(out=dst[64:96, :, fs], in_=hat[:])

                return [step(cc) for cc in range(CC)] + [fin]

            def tail_thunks(h, ctx_sb):
                def fin():
                    nc.gpsimd.dma_start(out=out_d[h], in_=ctx_sb[:])
                return [fin]

            def attn_section(h, ihalf, ctx_sb, fillq, prev_defer):
                """16-jc loop for one (head, i-half).  Even chunks: ACT
                exp->bf16 + bf16 ctx.  Odd chunks: fp8 pair tiles + DoubleRow
                ctx chains (v8 then vlo).  Tail work deferred to next section."""
                hp, par = divmod(h, 2)
                qa = aug_tiles[hp][par]
                ka = aug_tiles[hp][2 + par]
                st = {"started": False}
                ebs = [None] * (JC // 2)
                eps = [None] * (JC // 4)

                def ctx_mm(lhsT, rhs, last, perf_mode=None):
                    first = not st["started"]
                    st["started"] = True
                    for nn in range(2):
                        rr = (
                            rhs[:, nn * 512 : (nn + 1) * 512]
                            if perf_mode is None
                            else rhs[:, :, nn * 512 : (nn + 1) * 512]
                        )
                        nc.tensor.matmul(
                            st["cp"][0 : D + 1, nn * 512 : (nn + 1) * 512],
                            lhsT,
                            rr,
                            start=first,
                            stop=last,
                            perf_mode=perf_mode,
                        )

                def emit_ctx_b(c, last=False):
                    if "cp" not in st:
                        st["cp"] = psp.tile([128, 1024], f32, tag="ps", name="cp")
                    ctx_mm(vb[:, c // 2, h, 0 : D + 1], ebs[c // 2][:], last)

                def emit_ctx_a(q, last=False):
                    ctx_mm(v8[:, 2 * q : 2 * q + 2, h, 0 : D + 1], eps[q], False, DR)
                    ctx_mm(vlo[:, 2 * q : 2 * q + 2, h, 0 : D + 1], eps[q], last, DR)

                def final_copy():
                    nc.vector.tensor_copy(
                        ctx_sb[:, ihalf * 1024 : (ihalf + 1) * 1024],
                        st["cp"][0 : D + 1, :],
                    )

                for jc in range(JC):
                    if jc < len(prev_defer):
                        prev_defer[jc]()
                    sp = psp.tile([128, 1024], f32, tag="ps", name="sp")
                    for nn in range(2):
                        nc.tensor.matmul(
                            sp[:, nn * 512 : (nn + 1) * 512],
                            ka[:, :, jc * 128 : (jc + 1) * 128],
                            qa[
                                :,
                                :,
                                ihalf * 1024 + nn * 512 : ihalf * 1024 + (nn + 1) * 512,
                            ],
                            start=True,
                            stop=True,
                            perf_mode=DR,
                        )
                    if jc % 2 == 0:
                        ebs[jc // 2] = expb.tile([128, 1024], bf16, tag="expb", name="eb")
                        nc.scalar.activation(
                            ebs[jc // 2][:], sp[:], Exp, scale=1.0 / SCORE_SCALE
                        )
                    else:
                        a = (jc - 1) // 2
                        p, s = divmod(a, 2)
                        if s == 0:
                            eps[p] = expp.tile([128, 2, 1024], fp8, tag="expp", name="ep")
                        if jc in A_DVE_JCS:
                            nc.vector.tensor_scalar(
                                eps[p][:, s, :].bitcast(u8),
                                sp[:],
                                SCH_C1,
                                SCH_C2,
                                MUL,
                                ADD,
                            )
                        else:
                            nc.scalar.activation(
                                eps[p][:, s, :], sp[:], Exp, scale=1.0 / SCORE_SCALE
                            )
                    # lagged ctx: even chunk c at jc=c+2; fp8 pair q at jc=4q+5
                    if jc >= 2 and jc % 2 == 0:
                        emit_ctx_b(jc - 2)
                    elif jc >= 5 and (jc - 1) % 4 == 0:
                        emit_ctx_a((jc - 5) // 4)
                    if jc >= 2 and fillq:
                        fillq.pop(0)()
                        if fillq and jc % 2 == 0:
                            fillq.pop(0)()
                for th in fillq:
                    th()
                return [
                    (lambda: emit_ctx_b(JC - 2)),
                    (lambda: emit_ctx_a(JC // 4 - 1, last=True)),
                    final_copy,
                ]

            # ---- schedule ----
            make_aug(0)
            vths = v_thunks()
            pre = (
                proj_thunks(0, 0, 0)
                + proj_thunks(0, 1, 0)
                + [vths[jc] for jc in (0, 1, 2, 3, 4)]
            )
            for th in pre:
                th()

            ctx_sbs = {}
            defer = []
            pending_tail = []
            # fill order respects emission-order deadlines (a PE consumer must
            # never precede its producer in the PE stream)
            carry = (
                proj_thunks(0, 1, 1)
                + [vths[jc] for jc in (6, 8, 5, 7, 10, 12, 9, 11, 14, 13, 15)]
                + proj_thunks(0, 0, 1)
            )
            for hp in range(3):
                if hp + 1 < 3:
                    make_aug(hp + 1)
                    pj = []
                    for t in range(2):
                        for half in range(2):
                            pj += proj_thunks(hp + 1, t, half)
                else:
                    pj = []
                h_e, h_o = 2 * hp, 2 * hp + 1
                ctx_sbs[h_e] = ctxs.tile([D + 1, S], f32, tag="ctx_sb", name="ctx_sb")
                ctx_sbs[h_o] = ctxs.tile([D + 1, S], f32, tag="ctx_sb", name="ctx_sb")

                nfill = 21
                if hp == 0:
                    s0_fill = carry + pending_tail
                    s1_fill = pj[:nfill]
                    pj = pj[nfill:]
                else:
                    q0 = carry + pending_tail
                    s0_fill, rest = q0[:nfill], q0[nfill:]
                    s1_fill = rest + pj[: nfill - len(rest)]
                    pj = pj[nfill - len(rest) :]
                carry = []

                defer = attn_section(h_e, 0, ctx_sbs[h_e], s0_fill, defer)
                defer = attn_section(h_e, 1, ctx_sbs[h_e], s1_fill, defer)

                te = tail_thunks(h_e, ctx_sbs[h_e])
                s2_fill, pj = (te + pj)[:nfill], (te + pj)[nfill:]
                s3_fill = pj
                defer = attn_section(h_o, 0, ctx_sbs[h_o], s2_fill, defer)
                defer = attn_section(h_o, 1, ctx_sbs[h_o], s3_fill, defer)

                pending_tail = tail_thunks(h_o, ctx_sbs[h_o])

            for th in defer:
                th()
            for th in pending_tail:
                th()

    nc.finalize()
    return nc


def _get_nc(use_qk_bias: bool):
    key = ("prog", use_qk_bias)
    if key not in _CACHE:
        _CACHE[key] = _build_program(use_qk_bias)
    return _CACHE[key]


def _get_runner(use_qk_bias: bool):
    """Cached PJRT runner: one jitted executable + resident zero-output
    buffers; per call only the input blob is uploaded."""
    key = ("runner", use_qk_bias)
    if key in _CACHE:
        return _CACHE[key]

    nc = _get_nc(use_qk_bias)

    import jax
    from jax.sharding import Mesh, PartitionSpec
    from jax.experimental.shard_map import shard_map
    import concourse.mybir as mybir
    from concourse.bass2jax import (
        _bass_exec_p,
        install_neuronx_cc_hook,
        partition_id_tensor,
    )

    install_neuronx_cc_hook()
    partition_name = nc.partition_id_tensor.name if nc.partition_id_tensor else None
    in_names, out_names, out_avals, zero_outs = [], [], [], []
    for alloc in nc.m.functions[0].allocations:
        if not isinstance(alloc, mybir.MemoryLocationSet):
            continue
        name = alloc.memorylocations[0].name
        if alloc.kind == "ExternalInput":
            if name != partition_name:
                in_names.append(name)
        elif alloc.kind == "ExternalOutput":
            out_names.append(name)
            shape = tuple(alloc.tensor_shape)
            dtype = mybir.dt.np(alloc.dtype)
            out_avals.append(jax.core.ShapedArray(shape, dtype))
            zero_outs.append(np.zeros(shape, dtype))
    assert in_names == ["blob"] and out_names == ["out"]
    all_in_names = list(in_names) + list(out_names)
    if partition_name is not None:
        all_in_names.append(partition_name)

    def _body(*args):
        operands = list(args)
        if partition_name is not None:
            operands.append(partition_id_tensor())
        return tuple(
            _bass_exec_p.bind(
                *operands,
                out_avals=tuple(out_avals),
                in_names=tuple(all_in_names),
                out_names=tuple(out_names),
                lowering_input_output_aliases=(),
                sim_require_finite=True,
                sim_require_nnan=True,
                nc=nc,
            )
        )

    devices = jax.devices()[:NCORES]
    mesh = Mesh(np.asarray(devices), ("core",))
    sharded = jax.jit(
        shard_map(
            _body,
            mesh=mesh,
            in_specs=(PartitionSpec("core"),) * 2,
            out_specs=(PartitionSpec("core"),),
            check_rep=False,
        ),
        keep_unused=True,
    )
    dev_zeros = jax.device_put(
        np.zeros(
            (NCORES * zero_outs[0].shape[0], *zero_outs[0].shape[1:]),
            zero_outs[0].dtype,
        )
    )
    out_shape = tuple(out_avals[0].shape)

    def run(blobs: np.ndarray):
        (out,) = sharded(blobs, dev_zeros)
        return np.asarray(out).reshape(NCORES, *out_shape)

    _CACHE[key] = run
    return run


def _make_in_maps(hidden_states, Wq, bq, Wk, bk, Wv, bv, alphas, use_qk_bias):
    """Per-core packed blobs (uint8 1-D), as list of dicts keyed 'blob'."""
    import ml_dtypes

    bf = ml_dtypes.bfloat16
    G, _ = _cheb_factors(alphas)  # [12, 6, S] f32

    hs_b = []
    for b in range(B):
        t = np.ascontiguousarray(hidden_states[b].T)      # [768, S]
        t = t.reshape(CC, 128, S).transpose(1, 0, 2)      # [128, CC, S]
        hs_b.append(np.ascontiguousarray(t).astype(bf).view(np.uint8).ravel())

    w_hg, g_hg, bqk_hg = [], [], []
    for hg in range(2):
        rows = slice(hg * HGDIM, (hg + 1) * HGDIM)
        ws = []
        for W, s in ((Wq, SQ), (Wk, SK), (Wv, SV)):
            wt = np.ascontiguousarray(W[rows, :].T) * s       # [768, 384]
            wt = wt.reshape(CC, 128, HGDIM).transpose(1, 0, 2)
            ws.append(np.ascontiguousarray(wt).astype(bf).view(np.uint8).ravel())
        w_hg.append(np.concatenate(ws))
        gg = np.empty((3, 2, 9, 2, S), dtype=np.uint8)
        for hp in range(3):
            for par in range(2):
                Ghat, Gres = _bias_rows(SB * G[hg * HG + 2 * hp + par])
                gg[hp, par] = np.concatenate(
                    [
                        _fold35(Ghat.view(np.uint8)),
                        _fold35(Ghat.view(np.uint8)),
                        _fold35(Gres.view(np.uint8)),
                    ],
                    0,
                )
        g_hg.append(gg.ravel())
        bqk_hg.append(
            np.ascontiguousarray(
                np.stack([bq[rows] * SQ, bk[rows] * SK], 0), dtype=np.float32
            )
            .ravel()
            .view(np.uint8)
        )

    in_maps = []
    for core in range(NCORES):
        b, hg = divmod(core, 2)
        parts = [hs_b[b], w_hg[hg], g_hg[hg]]
        if use_qk_bias:
            parts.append(bqk_hg[hg])
        in_maps.append({"blob": np.concatenate(parts)})
    return in_maps


def kernel(hidden_states, Wq, bq, Wk, bk, Wv, bv, alphas):
    hidden_states = np.asarray(hidden_states, dtype=np.float32)
    Wq = np.asarray(Wq, dtype=np.float32)
    Wk = np.asarray(Wk, dtype=np.float32)
    Wv = np.asarray(Wv, dtype=np.float32)
    bq = np.asarray(bq, dtype=np.float32)
    bk = np.asarray(bk, dtype=np.float32)
    bv = np.asarray(bv, dtype=np.float32)
    alphas = np.asarray(alphas, dtype=np.float32)

    use_qk_bias = bool(np.any(bq) or np.any(bk))
    run = _get_runner(use_qk_bias)
    in_maps = _make_in_maps(
        hidden_states, Wq, bq, Wk, bk, Wv, bv, alphas, use_qk_bias
    )
    blobs = np.concatenate([m["blob"] for m in in_maps])
    res = run(blobs)  # [8, HG, 65, S] f32

    resf = np.asarray(res, dtype=np.float32)
    num = resf[:, :, 0:D, :]
    den = resf[:, :, D : D + 1, :]
    ctx = num / den
    ctx = ctx.transpose(0, 3, 1, 2).reshape(NCORES, S, HGDIM)

    out = np.empty((B, S, HIDDEN), dtype=np.float32)
    add_bv = bool(np.any(bv))
    for core in range(NCORES):
        b, hg = divmod(core, 2)
        o = ctx[core]
        if add_bv:
            o = o + bv[hg * HGDIM : (hg + 1) * HGDIM][None, :]
        out[b, :, hg * HGDIM : (hg + 1) * HGDIM] = o
    return out


# revision 38
# speedup vs baseline: 1.7205x; 1.2605x over previous
"""Chebyshev self-attention Trainium2 kernel (8-core SPMD).

Math restructuring
------------------
reference:  scores = (q @ k.T)/8 + cheb_bias(alphas)[h]  ;  softmax ; @ v

The Chebyshev bias factors exactly as bias_h[i,j] = sum_b G_h[i,b] u_j^b
(degree-5 polynomial in normalized positions); G is computed on the host
from `alphas` and shipped with the inputs, so the bias rides along as extra
contraction rows of the QK matmul.  No [S,S] bias tensor is materialized.

Precision / DoubleRow design (driven by a measured error budget vs the
2e-2 harness gate):

* projections run in bf16 (fp8 hidden-state/weight quantization alone costs
  4e-2 of output error - measured).
* scoresT[j,i] runs as ONE fp8e4 DoubleRow matmul per (j-chunk, i-half)
  with a 210-row contraction folded to [105 partitions, 2 slabs]:
      rows   0..63   q^ . k^          (fp8 of the bf16 projections)
      rows  64..127  q^ . kr          (kr = fp8 residual of k')
      rows 128..191  qr . k^          (qr = fp8 residual of q')
      rows 192..209  3-term bias:  G^.P^ + G^.pr + gr.P^
  The residual rows recover ~bf16 accuracy (the dropped qr.kr / gr.pr
  cross terms are ~0.1%) while the cost model charges a DoubleRow matmul
  half a bf16 one - contraction depth is free.  Residuals cost one DVE
  tensor_sub per projection epilogue; the q^/k^/bias row-blocks are laid
  into the folded aug tiles by free SBUF->SBUF partition-fold DMAs.
* ctx: per 16 j-chunks, the 8 EVEN chunks are "accurate": ACT exp->bf16
  and a bf16 matmul against bf16 v; the 8 ODD chunks pair into 4 fp8e4
  DoubleRow chains against fp8 v PLUS a second DoubleRow chain against the
  fp8 residual of v (fixes v-quantization, measured 1.7e-2 -> ~0.1e-2).
  exp for the odd chunks runs on ACT (exp->fp8) or on DVE as a Schraudolph
  bit-trick (u8(s_psum*8*log2e/256 + c) IS the e4m3 bit pattern of
  exp(s)); the per-section ACT/DVE split (N_DVE) is tuned so neither
  engine stalls in projection-heavy sections.  fp8-pathway probability
  noise averages over the 2048-term softmax sums.
* row 64 of each ctx accumulator is the softmax denominator (a constant
  4.0 column in every v tile); the [65, S] numerator/denominator go out in
  f32 and the host divides + transposes (no PE transposes at all).

Scheduling: DMAs that carry dependencies (partition folds, G/P rows,
outputs) issue from the Pool engine's software DGE so they never hold the
SP/ACT queues through their waits; V-projection and next-head-pair
projection work is drip-fed into the attention sections' fill queues with
emission-order deadlines (a PE consumer must never precede its producer
in the PE stream).  Engine busy (cost model): PE ~153us, ACT ~149us,
DVE ~144us; simulated total ~226us vs the 269us f32r baseline.
"""

import numpy as np
from math import comb

B = 4
S = 2048
HIDDEN = 768
HEADS = 12
D = 64
ORDER = 5
NCORES = 8
HG = HEADS // 2          # heads per core (6)
HGDIM = HG * D           # 384 output columns per core
CC = HIDDEN // 128       # contraction chunks (6)
JC = S // 128            # j tiles (16)
VP = 80                  # padded per-head fp8 v row (64 data + 1 denom + pad)
VPB = 66                 # padded per-head bf16 v row
NAUGP = 105              # folded aug partitions (210 rows / 2)

# per-section j-chunk plan: even chunks accurate (ACT exp->bf16, bf16 ctx),
# odd chunks fp8 DoubleRow.  Per section, the first n_dve entries of
# DVE_ORDER run their exp on DVE (Schraudolph); the rest on ACT (exp->fp8).
# Sections that carry projection fill work get fewer DVE exps.
DVE_ORDER = (3, 7, 11, 15, 5, 9, 13, 1)
N_DVE = (0, 0, 6, 7, 7, 3, 3, 7, 7, 3, 3, 8)

# scale bookkeeping
SQ = 4.0                 # q' = SQ * q
SK = 8.0                 # k' = SK * k   (SQ*SK = 256/8 -> psum = 256*scores)
SB = 16.0                # G' = SB*G, P' = SB*u^b
SV = 4.0                 # v' = SV * v ; denominator column = SV
SCORE_SCALE = 256.0

# blob layout (uint8 offsets, per core)
OFF_HS = 0
N_HS = HIDDEN * S * 2                  # [128, CC, S] bf16
OFF_W = OFF_HS + N_HS
N_W = 3 * HIDDEN * HGDIM * 2           # [3, 128, CC, HGDIM] bf16
OFF_G = OFF_W + N_W
N_G = 3 * 2 * 9 * 2 * S                # [hp, par, 9, 2, S] fp8
OFF_BQK = OFF_G + N_G
N_BQK = 2 * HGDIM * 4                  # [2, 384] f32 raw bytes
NB_NOBIAS = OFF_BQK
NB_BIAS = OFF_BQK + N_BQK

_CACHE = {}


def _cheb_factors(alphas: np.ndarray):
    """alphas [H, 6] -> G [H, 6, S] (i-side, f32), P [6, S] (j-side, f32)."""
    import numpy.polynomial.chebyshev as cheb

    T = np.zeros((ORDER + 1, ORDER + 1))
    for k in range(ORDER + 1):
        e = np.zeros(k + 1)
        e[k] = 1
        T[k, : k + 1] = cheb.cheb2poly(e)[: k + 1]
    c = alphas.astype(np.float64) @ T
    v = np.arange(S, dtype=np.float64) / (S - 1)
    G = np.zeros((HEADS, ORDER + 1, S))
    for h in range(HEADS):
        for b in range(ORDER + 1):
            acc = np.zeros(S)
            for a in range(0, ORDER + 1 - b):
                acc += c[h, a + b] * comb(a + b, a) * ((-v) ** a)
            G[h, b, :] = acc
    P = np.stack([v**b for b in range(ORDER + 1)], 0)
    return G.astype(np.float32), P.astype(np.float32)


def _np_fp8():
    import concourse.mybir as mybir

    return mybir.dt.np(mybir.dt.float8e4)


def _fold35(x6: np.ndarray) -> np.ndarray:
    """[6, S] rows -> folded [3, 2, S] (row 2a+s -> (a, s))."""
    return np.ascontiguousarray(x6.reshape(3, 2, -1))


def _bias_rows(x6: np.ndarray) -> tuple[np.ndarray, np.ndarray]:
    """[6, S] f32 -> (hat [6,S] fp8->f32, residual fp8 bytes) pair."""
    fp8 = _np_fp8()
    hat8 = np.clip(x6, -240, 240).astype(fp8)
    res8 = np.clip(x6 - hat8.astype(np.float32), -240, 240).astype(fp8)
    return hat8, res8


def _build_program(use_qk_bias: bool):
    import concourse.bass as bass
    import concourse.mybir as mybir
    import concourse.tile as tile
    from concourse import bacc

    f32 = mybir.dt.float32
    bf16 = mybir.dt.bfloat16
    fp8 = mybir.dt.float8e4
    u8 = mybir.dt.uint8
    Exp = mybir.ActivationFunctionType.Exp
    Ident = mybir.ActivationFunctionType.Identity
    DR = mybir.MatmulPerfMode.DoubleRow
    MUL = mybir.AluOpType.mult
    ADD = mybir.AluOpType.add

    # Schraudolph constants: e4m3 bits of exp(s) from psum value 256*s
    SCH_C1 = 8.0 * np.log2(np.e) / SCORE_SCALE
    SCH_C2 = 56.0 - 0.5 * 0.458

    nc = bacc.Bacc("TRN2", target_bir_lowering=False, debug=False)

    nb = NB_BIAS if use_qk_bias else NB_NOBIAS
    blob_d = nc.dram_tensor("blob", [nb], u8, kind="ExternalInput")
    out_d = nc.dram_tensor("out", [HG, D + 1, S], f32, kind="ExternalOutput")

    blob = blob_d.ap()
    hs_d = blob[OFF_HS : OFF_HS + N_HS].rearrange(
        "(p c s two) -> p c (s two)", c=CC, s=S, two=2
    )
    w_d = blob[OFF_W : OFF_W + N_W].rearrange(
        "(t p c n two) -> t p c (n two)", t=3, p=128, c=CC, n=HGDIM, two=2
    )
    g_d = blob[OFF_G : OFF_G + N_G].rearrange(
        "(hp par a s i) -> hp par a s i", hp=3, par=2, a=9, s=2, i=S
    )

    # j-side bias rows (fp8): [P^ ; pr ; P^] folded -> [9, 2, S]
    pos = np.arange(S, dtype=np.float64) / (S - 1)
    Pp = (np.stack([(pos**b) for b in range(ORDER + 1)], 0) * SB).astype(np.float32)
    Phat, Pres = _bias_rows(Pp)
    pT9 = np.concatenate(
        [_fold35(Phat.view(np.uint8)), _fold35(Pres.view(np.uint8)), _fold35(Phat.view(np.uint8))], 0
    )
    pT_d = nc.inline_tensor(pT9, name="pT9")

    with tile.TileContext(nc) as tc:
        import contextlib

        with contextlib.ExitStack() as ctx:
            consts = ctx.enter_context(tc.tile_pool(name="consts", bufs=1))
            hsT = consts.tile([128, CC, S], bf16, name="hsT")
            w_sb = [consts.tile([128, CC, HGDIM], bf16, name=f"w{t}") for t in range(3)]
            nc.sync.dma_start(out=w_sb[0][:].bitcast(u8), in_=w_d[0])
            nc.sync.dma_start(out=hsT[:, 0, :].bitcast(u8), in_=hs_d[:, 0, :])
            nc.sync.dma_start(out=hsT[:, 1, :].bitcast(u8), in_=hs_d[:, 1, :])
            nc.sync.dma_start(out=w_sb[1][:].bitcast(u8), in_=w_d[1])
            nc.sync.dma_start(out=hsT[:, 2, :].bitcast(u8), in_=hs_d[:, 2, :])
            nc.sync.dma_start(out=w_sb[2][:].bitcast(u8), in_=w_d[2])
            for cc in range(3, CC):
                nc.sync.dma_start(out=hsT[:, cc, :].bitcast(u8), in_=hs_d[:, cc, :])
            v8 = consts.tile([128, JC // 2, HG, VP], fp8, name="v8")
            vlo = consts.tile([128, JC // 2, HG, VP], fp8, name="vlo")
            vb = consts.tile([128, JC // 2, HG, VPB], bf16, name="vb")
            nc.vector.memset(v8[:, :, :, D : D + 1], SV)
            nc.vector.memset(vlo[:, :, :, D : D + 1], 0.0)
            nc.vector.memset(vb[:, :, :, D : D + 1], SV)
            if use_qk_bias:
                bqk_d = blob[OFF_BQK : OFF_BQK + N_BQK].rearrange(
                    "(t hp p four) -> p t hp four", t=2, hp=3, p=128, four=4
                )
                bqk = consts.tile([128, 2, 3], f32)
                nc.sync.dma_start(out=bqk[:].bitcast(u8), in_=bqk_d)

            aug = ctx.enter_context(tc.tile_pool(name="aug", bufs=2))
            psp = ctx.enter_context(tc.tile_pool(name="psp", bufs=4, space="PSUM"))
            expp = ctx.enter_context(tc.tile_pool(name="expp", bufs=4))
            expb = ctx.enter_context(tc.tile_pool(name="expb", bufs=4))
            stage = ctx.enter_context(tc.tile_pool(name="stage", bufs=8))
            ctxs = ctx.enter_context(tc.tile_pool(name="ctxs", bufs=2))

            # ---- V projection (bf16): v' for all 6 heads; fp8 + residual +
            # bf16 copies per chunk parity ----
            def v_thunks():
                ths = []

                def mk(jc):
                    def f():
                        vp = psp.tile([128, 1024], f32, tag="ps", name="vp")
                        for cc in range(CC):
                            nc.tensor.matmul(
                                vp[:, 0:HGDIM],
                                hsT[:, cc, jc * 128 : (jc + 1) * 128],
                                w_sb[2][:, cc, :],
                                start=(cc == 0),
                                stop=(cc == CC - 1),
                            )
                        vph = vp[:, 0:HGDIM].rearrange("p (h d) -> p h d", d=D)
                        if jc % 2 == 0:
                            nc.vector.tensor_copy(vb[:, jc // 2, :, 0:D], vph)
                        else:
                            a = (jc - 1) // 2
                            nc.vector.tensor_copy(v8[:, a, :, 0:D], vph)
                            nc.vector.tensor_sub(
                                out=vlo[:, a, :, 0:D], in0=vph, in1=v8[:, a, :, 0:D]
                            )
                    return f

                for jc in range(JC):
                    ths.append(mk(jc))
                return ths

            aug_tiles = {}

            def make_aug(hp):
                # folded aug [105, 2, S]: parts 0..31 q^/k^, 32..63 q^/kr,
                # 64..95 qr/k^, 96..104 bias rows
                qe = aug.tile([NAUGP, 2, S], fp8, tag="qaug_e")
                qo = aug.tile([NAUGP, 2, S], fp8, tag="qaug_o")
                ke = aug.tile([NAUGP, 2, S], fp8, tag="kaug_e")
                ko = aug.tile([NAUGP, 2, S], fp8, tag="kaug_o")
                nc.gpsimd.dma_start(out=qe[96:105].bitcast(u8), in_=g_d[hp, 0])
                nc.gpsimd.dma_start(out=qo[96:105].bitcast(u8), in_=g_d[hp, 1])
                nc.gpsimd.dma_start(out=ke[96:105].bitcast(u8), in_=pT_d[:])
                nc.gpsimd.dma_start(out=ko[96:105].bitcast(u8), in_=pT_d[:])
                aug_tiles[hp] = (qe, qo, ke, ko)

            def proj_thunks(hp, t, half, act_hat=False):
                """One (tensor, i-half) bf16 projection: 6 cc steps x2 nn into
                a PSUM tile; epilogue: fp8 hat copies (DVE, or ACT at startup),
                fp8 residual subs (DVE), and 6 partition-fold DMAs (split
                across the Pool-SWDGE and SP-HWDGE queues)."""
                qe, qo, ke, ko = aug_tiles[hp]
                dst_e, dst_o = (qe, qo) if t == 0 else (ke, ko)
                st = {}

                def step(cc):
                    def f():
                        if cc == 0:
                            st["pp"] = psp.tile([128, 1024], f32, tag="ps", name="pp")
                        pp = st["pp"]
                        for nn in range(2):
                            nc.tensor.matmul(
                                pp[:, nn * 512 : (nn + 1) * 512],
                                w_sb[t][:, cc, hp * 128 : (hp + 1) * 128],
                                hsT[
                                    :,
                                    cc,
                                    half * 1024 + nn * 512 : half * 1024 + (nn + 1) * 512,
                                ],
                                start=(cc == 0),
                                stop=(cc == CC - 1),
                            )
                    return f

                def fin(par, dst):
                    def f():
                        pp = st["pp"]
                        fs = slice(half * 1024, (half + 1) * 1024)
                        ps = pp[0:D, :] if par == 0 else pp[D:128, :]
                        hat = stage.tile([64, 1024], fp8, tag="stage", name="hat")
                        res = stage.tile([64, 1024], fp8, tag="stage", name="res")
                        if use_qk_bias:
                            nc.scalar.activation(
                                hat[:], ps, Ident, bias=bqk[par * D : (par + 1) * D, t, hp]
                            )
                            nc.vector.memset(res[:], 0.0)
                        elif act_hat:
                            nc.scalar.activation(hat[:], ps, Ident)
                            nc.vector.tensor_sub(out=res[:], in0=ps, in1=hat[:])
                        else:
                            nc.vector.tensor_copy(hat[:], ps)
                            nc.vector.tensor_sub(out=res[:], in0=ps, in1=hat[:])
                        eng3 = nc.scalar if act_hat else nc.gpsimd
                        if t == 0:  # q-side: q^, q^, qr
                            nc.gpsimd.dma_start(out=dst[0:32, :, fs], in_=hat[:])
                            nc.sync.dma_start(out=dst[32:64, :, fs], in_=hat[:])
                            eng3.dma_start(out=dst[64:96, :, fs], in_=res[:])
                        else:  # k-side: k^, kr, k^
                            nc.sync.dma_start(out=dst[0:32, :, fs], in_=hat[:])
                            eng3.dma_start(out=dst[32:64, :, fs], in_=res[:])
                            nc.gpsimd.dma_start(out=dst[64:96, :, fs], in_=hat[:])
                    return f

                return [step(cc) for cc in range(CC)] + [fin(0, dst_e), fin(1, dst_o)]

            def tail_thunks(h, ctx_sb):
                return []

            def attn_section(
                h, ihalf, ctx_sb, fillq, prev_defer, sec_idx=0, last=False
            ):
                """16-jc loop for one (head, i-half).  Even chunks: ACT
                exp->bf16 + bf16 ctx.  Odd chunks: fp8 pair tiles + DoubleRow
                ctx chains (v8 then vlo).  Tail work deferred to next section."""
                hp, par = divmod(h, 2)
                qa = aug_tiles[hp][par]
                ka = aug_tiles[hp][2 + par]
                st = {"started": False}
                ebs = [None] * (JC // 2)
                eps = [None] * (JC // 4)
                dve_jcs = set(DVE_ORDER[: N_DVE[sec_idx]])

                def ctx_mm(lhsT, rhs, last, perf_mode=None):
                    first = not st["started"]
                    st["started"] = True
                    for nn in range(2):
                        rr = (
                            rhs[:, nn * 512 : (nn + 1) * 512]
                            if perf_mode is None
                            else rhs[:, :, nn * 512 : (nn + 1) * 512]
                        )
                        nc.tensor.matmul(
                            st["cp"][0 : D + 1, nn * 512 : (nn + 1) * 512],
                            lhsT,
                            rr,
                            start=first,
                            stop=last,
                            perf_mode=perf_mode,
                        )

                def emit_ctx_b(c, last=False):
                    if "cp" not in st:
                        st["cp"] = psp.tile([128, 1024], f32, tag="ps", name="cp")
                    ctx_mm(vb[:, c // 2, h, 0 : D + 1], ebs[c // 2][:], last)

                def emit_ctx_a(q, last=False):
                    ctx_mm(v8[:, 2 * q : 2 * q + 2, h, 0 : D + 1], eps[q], False, DR)
                    ctx_mm(vlo[:, 2 * q : 2 * q + 2, h, 0 : D + 1], eps[q], last, DR)

                def final_copy():
                    nc.vector.tensor_copy(
                        ctx_sb[:, ihalf * 1024 : (ihalf + 1) * 1024],
                        st["cp"][0 : D + 1, :],
                    )
                    nc.gpsimd.dma_start(
                        out=out_d[h][:, ihalf * 1024 : (ihalf + 1) * 1024],
                        in_=ctx_sb[:, ihalf * 1024 : (ihalf + 1) * 1024],
                    )

                for jc in range(JC):
                    if jc < len(prev_defer):
                        prev_defer[jc]()
                    sp = psp.tile([128, 1024], f32, tag="ps", name="sp")
                    for nn in range(2):
                        nc.tensor.matmul(
                            sp[:, nn * 512 : (nn + 1) * 512],
                            ka[:, :, jc * 128 : (jc + 1) * 128],
                            qa[
                                :,
                                :,
                                ihalf * 1024 + nn * 512 : ihalf * 1024 + (nn + 1) * 512,
                            ],
                            start=True,
                            stop=True,
                            perf_mode=DR,
                        )
                    if jc % 2 == 0:
                        ebs[jc // 2] = expb.tile([128, 1024], bf16, tag="expb", name="eb")
                        nc.scalar.activation(
                            ebs[jc // 2][:], sp[:], Exp, scale=1.0 / SCORE_SCALE
                        )
                    else:
                        a = (jc - 1) // 2
                        p, s = divmod(a, 2)
                        if s == 0:
                            eps[p] = expp.tile([128, 2, 1024], fp8, tag="expp", name="ep")
                        if jc in dve_jcs:
                            nc.vector.tensor_scalar(
                                eps[p][:, s, :].bitcast(u8),
                                sp[:],
                                SCH_C1,
                                SCH_C2,
                                MUL,
                                ADD,
                            )
                        else:
                            nc.scalar.activation(
                                eps[p][:, s, :], sp[:], Exp, scale=1.0 / SCORE_SCALE
                            )
                    if jc >= 2:
                        for _ in range(2):
                            if fillq:
                                fillq.pop(0)()
                    # lagged ctx: even chunk c at jc=c+2; fp8 pair q at jc=4q+5
                    if jc >= 2 and jc % 2 == 0:
                        emit_ctx_b(jc - 2)
                    elif jc >= 5 and (jc - 1) % 4 == 0:
                        emit_ctx_a((jc - 5) // 4)
                for th in fillq:
                    th()
                ret = [
                    (lambda: emit_ctx_b(JC - 2)),
                    (lambda: emit_ctx_a(JC // 4 - 1, last=True)),
                    final_copy,
                ]
                if last:
                    for th in ret:
                        th()
                    ret = []
                return ret

            # ---- schedule ----
            make_aug(0)
            vths = v_thunks()
            pre = (
                proj_thunks(0, 0, 0, act_hat=True)
                + proj_thunks(0, 1, 0, act_hat=True)
                + [vths[jc] for jc in (0, 1, 2, 3, 4)]
            )
            for th in pre:
                th()

            ctx_sbs = {}
            defer = []
            pending_tail = []
            # fill order respects emission-order deadlines (a PE consumer must
            # never precede its producer in the PE stream)
            carry = (
                proj_thunks(0, 1, 1)
                + [vths[jc] for jc in range(5, JC)]
                + proj_thunks(0, 0, 1)
            )
            for hp in range(3):
                if hp + 1 < 3:
                    make_aug(hp + 1)
                    pj = []
                    for t in range(2):
                        for half in range(2):
                            pj += proj_thunks(hp + 1, t, half)
                else:
                    pj = []
                h_e, h_o = 2 * hp, 2 * hp + 1
                ctx_sbs[h_e] = ctxs.tile([D + 1, S], f32, tag="ctx_sb", name="ctx_sb")
                ctx_sbs[h_o] = ctxs.tile([D + 1, S], f32, tag="ctx_sb", name="ctx_sb")

                nfill = 28
                if hp == 0:
                    s0_fill = carry + pending_tail
                    s1_fill = pj[:nfill]
                    pj = pj[nfill:]
                else:
                    q0 = carry + pending_tail
                    s0_fill, rest = q0[:nfill], q0[nfill:]
                    s1_fill = rest + pj[: nfill - len(rest)]
                    pj = pj[nfill - len(rest) :]
                carry = []

                defer = attn_section(
                    h_e, 0, ctx_sbs[h_e], s0_fill, defer, sec_idx=4 * hp
                )
                defer = attn_section(
                    h_e, 1, ctx_sbs[h_e], s1_fill, defer, sec_idx=4 * hp + 1
                )

                te = tail_thunks(h_e, ctx_sbs[h_e])
                s2_fill, pj = (te + pj)[:nfill], (te + pj)[nfill:]
                s3_fill = pj
                defer = attn_section(
                    h_o, 0, ctx_sbs[h_o], s2_fill, defer, sec_idx=4 * hp + 2
                )
                defer = attn_section(
                    h_o,
                    1,
                    ctx_sbs[h_o],
                    s3_fill,
                    defer,
                    sec_idx=4 * hp + 3,
                    last=(hp == 2),
                )

                pending_tail = tail_thunks(h_o, ctx_sbs[h_o])

            for th in defer:
                th()
            for th in pending_tail:
                th()

    nc.finalize()
    return nc


def _get_nc(use_qk_bias: bool):
    key = ("prog", use_qk_bias)
    if key not in _CACHE:
        _CACHE[key] = _build_program(use_qk_bias)
    return _CACHE[key]


def _get_runner(use_qk_bias: bool):
    """Cached PJRT runner: one jitted executable + resident zero-output
    buffers; per call only the input blob is uploaded."""
    key = ("runner", use_qk_bias)
    if key in _CACHE:
        return _CACHE[key]

    nc = _get_nc(use_qk_bias)

    import jax
    from jax.sharding import Mesh, PartitionSpec
    from jax.experimental.shard_map import shard_map
    import concourse.mybir as mybir
    from concourse.bass2jax import (
        _bass_exec_p,
        install_neuronx_cc_hook,
        partition_id_tensor,
    )

    install_neuronx_cc_hook()
    partition_name = nc.partition_id_tensor.name if nc.partition_id_tensor else None
    in_names, out_names, out_avals, zero_outs = [], [], [], []
    for alloc in nc.m.functions[0].allocations:
        if not isinstance(alloc, mybir.MemoryLocationSet):
            continue
        name = alloc.memorylocations[0].name
        if alloc.kind == "ExternalInput":
            if name != partition_name:
                in_names.append(name)
        elif alloc.kind == "ExternalOutput":
            out_names.append(name)
            shape = tuple(alloc.tensor_shape)
            dtype = mybir.dt.np(alloc.dtype)
            out_avals.append(jax.core.ShapedArray(shape, dtype))
            zero_outs.append(np.zeros(shape, dtype))
    assert in_names == ["blob"] and out_names == ["out"]
    all_in_names = list(in_names) + list(out_names)
    if partition_name is not None:
        all_in_names.append(partition_name)

    def _body(*args):
        operands = list(args)
        if partition_name is not None:
            operands.append(partition_id_tensor())
        return tuple(
            _bass_exec_p.bind(
                *operands,
                out_avals=tuple(out_avals),
                in_names=tuple(all_in_names),
                out_names=tuple(out_names),
                lowering_input_output_aliases=(),
                sim_require_finite=True,
                sim_require_nnan=True,
                nc=nc,
            )
        )

    devices = jax.devices()[:NCORES]
    mesh = Mesh(np.asarray(devices), ("core",))
    sharded = jax.jit(
        shard_map(
            _body,
            mesh=mesh,
            in_specs=(PartitionSpec("core"),) * 2,
            out_specs=(PartitionSpec("core"),),
            check_rep=False,
        ),
        keep_unused=True,
    )
    dev_zeros = jax.device_put(
        np.zeros(
            (NCORES * zero_outs[0].shape[0], *zero_outs[0].shape[1:]),
            zero_outs[0].dtype,
        )
    )
    out_shape = tuple(out_avals[0].shape)

    def run(blobs: np.ndarray):
        (out,) = sharded(blobs, dev_zeros)
        return np.asarray(out).reshape(NCORES, *out_shape)

    _CACHE[key] = run
    return run


def _make_in_maps(hidden_states, Wq, bq, Wk, bk, Wv, bv, alphas, use_qk_bias):
    """Per-core packed blobs (uint8 1-D), as list of dicts keyed 'blob'."""
    import ml_dtypes

    bf = ml_dtypes.bfloat16
    G, _ = _cheb_factors(alphas)  # [12, 6, S] f32

    hs_b = []
    for b in range(B):
        t = np.ascontiguousarray(hidden_states[b].T)      # [768, S]
        t = t.reshape(CC, 128, S).transpose(1, 0, 2)      # [128, CC, S]
        hs_b.append(np.ascontiguousarray(t).astype(bf).view(np.uint8).ravel())

    w_hg, g_hg, bqk_hg = [], [], []
    for hg in range(2):
        rows = slice(hg * HGDIM, (hg + 1) * HGDIM)
        ws = []
        for W, s in ((Wq, SQ), (Wk, SK), (Wv, SV)):
            wt = np.ascontiguousarray(W[rows, :].T) * s       # [768, 384]
            wt = wt.reshape(CC, 128, HGDIM).transpose(1, 0, 2)
            ws.append(np.ascontiguousarray(wt).astype(bf).view(np.uint8).ravel())
        w_hg.append(np.concatenate(ws))
        gg = np.empty((3, 2, 9, 2, S), dtype=np.uint8)
        for hp in range(3):
            for par in range(2):
                Ghat, Gres = _bias_rows(SB * G[hg * HG + 2 * hp + par])
                gg[hp, par] = np.concatenate(
                    [
                        _fold35(Ghat.view(np.uint8)),
                        _fold35(Ghat.view(np.uint8)),
                        _fold35(Gres.view(np.uint8)),
                    ],
                    0,
                )
        g_hg.append(gg.ravel())
        bqk_hg.append(
            np.ascontiguousarray(
                np.stack([bq[rows] * SQ, bk[rows] * SK], 0), dtype=np.float32
            )
            .ravel()
            .view(np.uint8)
        )

    in_maps = []
    for core in range(NCORES):
        b, hg = divmod(core, 2)
        parts = [hs_b[b], w_hg[hg], g_hg[hg]]
        if use_qk_bias:
            parts.append(bqk_hg[hg])
        in_maps.append({"blob": np.concatenate(parts)})
    return in_maps


def kernel(hidden_states, Wq, bq, Wk, bk, Wv, bv, alphas):
    hidden_states = np.asarray(hidden_states, dtype=np.float32)
    Wq = np.asarray(Wq, dtype=np.float32)
    Wk = np.asarray(Wk, dtype=np.float32)
    Wv = np.asarray(Wv, dtype=np.float32)
    bq = np.asarray(bq, dtype=np.float32)
    bk = np.asarray(bk, dtype=np.float32)
    bv = np.asarray(bv, dtype=np.float32)
    alphas = np.asarray(alphas, dtype=np.float32)

    use_qk_bias = bool(np.any(bq) or np.any(bk))
    run = _get_runner(use_qk_bias)
    in_maps = _make_in_maps(
        hidden_states, Wq, bq, Wk, bk, Wv, bv, alphas, use_qk_bias
    )
    blobs = np.concatenate([m["blob"] for m in in_maps])
    res = run(blobs)  # [8, HG, 65, S] f32

    resf = np.asarray(res, dtype=np.float32)
    num = resf[:, :, 0:D, :]
    den = resf[:, :, D : D + 1, :]
    ctx = num / den
    ctx = ctx.transpose(0, 3, 1, 2).reshape(NCORES, S, HGDIM)

    out = np.empty((B, S, HIDDEN), dtype=np.float32)
    add_bv = bool(np.any(bv))
    for core in range(NCORES):
        b, hg = divmod(core, 2)
        o = ctx[core]
        if add_bv:
            o = o + bv[hg * HGDIM : (hg + 1) * HGDIM][None, :]
        out[b, :, hg * HGDIM : (hg + 1) * HGDIM] = o
    return out


# revision 47
# speedup vs baseline: 1.7603x; 1.0231x over previous
"""Chebyshev self-attention Trainium2 kernel (8-core SPMD).

Math restructuring
------------------
reference:  scores = (q @ k.T)/8 + cheb_bias(alphas)[h]  ;  softmax ; @ v

The Chebyshev bias factors exactly as bias_h[i,j] = sum_b G_h[i,b] u_j^b
(degree-5 polynomial in normalized positions); G is computed on the host
from `alphas` and shipped with the inputs, so the bias rides along as extra
contraction rows of the QK matmul.  No [S,S] bias tensor is materialized.

Precision / DoubleRow design (driven by a measured error budget vs the
2e-2 harness gate):

* projections run in bf16 (fp8 hidden-state/weight quantization alone costs
  4e-2 of output error - measured).
* scoresT[j,i] runs as ONE fp8e4 DoubleRow matmul per (j-chunk, i-half)
  with a 210-row contraction folded to [105 partitions, 2 slabs]:
      rows   0..63   q^ . k^          (fp8 of the bf16 projections)
      rows  64..127  q^ . kr          (kr = fp8 residual of k')
      rows 128..191  qr . k^          (qr = fp8 residual of q')
      rows 192..209  3-term bias:  G^.P^ + G^.pr + gr.P^
  The residual rows recover ~bf16 accuracy (the dropped qr.kr / gr.pr
  cross terms are ~0.1%) while the cost model charges a DoubleRow matmul
  half a bf16 one - contraction depth is free.  Residuals cost one DVE
  tensor_sub per projection epilogue; the q^/k^/bias row-blocks are laid
  into the folded aug tiles by free SBUF->SBUF partition-fold DMAs.
* ctx: per 16 j-chunks, the 8 EVEN chunks are "accurate": ACT exp->bf16
  and a bf16 matmul against bf16 v; the 8 ODD chunks pair into 4 fp8e4
  DoubleRow chains against fp8 v PLUS a second DoubleRow chain against the
  fp8 residual of v (fixes v-quantization, measured 1.7e-2 -> ~0.1e-2).
  exp for the odd chunks runs on ACT (exp->fp8) or on DVE as a Schraudolph
  bit-trick (u8(s_psum*8*log2e/256 + c) IS the e4m3 bit pattern of
  exp(s)); the per-section ACT/DVE split (N_DVE) is tuned so neither
  engine stalls in projection-heavy sections.  fp8-pathway probability
  noise averages over the 2048-term softmax sums.
* row 64 of each ctx accumulator is the softmax denominator (a constant
  4.0 column in every v tile); the [65, S] numerator/denominator go out in
  f32 and the host divides + transposes (no PE transposes at all).

Scheduling: DMAs that carry dependencies (partition folds, G/P rows,
outputs) issue from the Pool engine's software DGE so they never hold the
SP/ACT queues through their waits; V-projection and next-head-pair
projection work is drip-fed into the attention sections' fill queues with
emission-order deadlines (a PE consumer must never precede its producer
in the PE stream).  Engine busy (cost model): PE ~153us, ACT ~149us,
DVE ~144us; simulated total ~226us vs the 269us f32r baseline.
"""

import numpy as np
from math import comb

B = 4
S = 2048
HIDDEN = 768
HEADS = 12
D = 64
ORDER = 5
NCORES = 8
HG = HEADS // 2          # heads per core (6)
HGDIM = HG * D           # 384 output columns per core
CC = HIDDEN // 128       # contraction chunks (6)
JC = S // 128            # j tiles (16)
VP = 80                  # padded per-head fp8 v row (64 data + 1 denom + pad)
VPB = 66                 # padded per-head bf16 v row
NAUGP = 105              # folded aug partitions (210 rows / 2)

# per-section j-chunk plan: even chunks accurate (ACT exp->bf16, bf16 ctx),
# odd chunks fp8 DoubleRow.  Per section, the first n_dve entries of
# DVE_ORDER run their exp on DVE (Schraudolph); the rest on ACT (exp->fp8).
# Sections that carry projection fill work get fewer DVE exps.
DVE_ORDER = (3, 7, 11, 15, 5, 9, 13, 1)
N_DVE = (0, 1, 6, 7, 7, 3, 3, 7, 7, 3, 3, 8)

# scale bookkeeping
SQ = 4.0                 # q' = SQ * q
SK = 8.0                 # k' = SK * k   (SQ*SK = 256/8 -> psum = 256*scores)
SB = 16.0                # G' = SB*G, P' = SB*u^b
SV = 4.0                 # v' = SV * v ; denominator column = SV
SCORE_SCALE = 256.0

# blob layout (uint8 offsets, per core)
OFF_HS = 0
N_HS = HIDDEN * S * 2                  # [128, CC, S] bf16
OFF_W = OFF_HS + N_HS
N_W = 3 * HIDDEN * HGDIM * 2           # [3, 128, CC, HGDIM] bf16
OFF_G = OFF_W + N_W
N_G = 3 * 2 * 9 * 2 * S                # [hp, par, 9, 2, S] fp8
OFF_BQK = OFF_G + N_G
N_BQK = 2 * HGDIM * 4                  # [2, 384] f32 raw bytes
NB_NOBIAS = OFF_BQK
NB_BIAS = OFF_BQK + N_BQK

_CACHE = {}


def _cheb_factors(alphas: np.ndarray):
    """alphas [H, 6] -> G [H, 6, S] (i-side, f32), P [6, S] (j-side, f32)."""
    import numpy.polynomial.chebyshev as cheb

    T = np.zeros((ORDER + 1, ORDER + 1))
    for k in range(ORDER + 1):
        e = np.zeros(k + 1)
        e[k] = 1
        T[k, : k + 1] = cheb.cheb2poly(e)[: k + 1]
    c = alphas.astype(np.float64) @ T
    v = np.arange(S, dtype=np.float64) / (S - 1)
    G = np.zeros((HEADS, ORDER + 1, S))
    for h in range(HEADS):
        for b in range(ORDER + 1):
            acc = np.zeros(S)
            for a in range(0, ORDER + 1 - b):
                acc += c[h, a + b] * comb(a + b, a) * ((-v) ** a)
            G[h, b, :] = acc
    P = np.stack([v**b for b in range(ORDER + 1)], 0)
    return G.astype(np.float32), P.astype(np.float32)


def _np_fp8():
    import concourse.mybir as mybir

    return mybir.dt.np(mybir.dt.float8e4)


def _fold35(x6: np.ndarray) -> np.ndarray:
    """[6, S] rows -> folded [3, 2, S] (row 2a+s -> (a, s))."""
    return np.ascontiguousarray(x6.reshape(3, 2, -1))


def _bias_rows(x6: np.ndarray) -> tuple[np.ndarray, np.ndarray]:
    """[6, S] f32 -> (hat [6,S] fp8->f32, residual fp8 bytes) pair."""
    fp8 = _np_fp8()
    hat8 = np.clip(x6, -240, 240).astype(fp8)
    res8 = np.clip(x6 - hat8.astype(np.float32), -240, 240).astype(fp8)
    return hat8, res8


def _build_program(use_qk_bias: bool):
    import concourse.bass as bass
    import concourse.mybir as mybir
    import concourse.tile as tile
    from concourse import bacc

    f32 = mybir.dt.float32
    bf16 = mybir.dt.bfloat16
    fp8 = mybir.dt.float8e4
    u8 = mybir.dt.uint8
    Exp = mybir.ActivationFunctionType.Exp
    Ident = mybir.ActivationFunctionType.Identity
    DR = mybir.MatmulPerfMode.DoubleRow
    MUL = mybir.AluOpType.mult
    ADD = mybir.AluOpType.add

    # Schraudolph constants: e4m3 bits of exp(s) from psum value 256*s
    SCH_C1 = 8.0 * np.log2(np.e) / SCORE_SCALE
    SCH_C2 = 56.0 - 0.5 * 0.458

    nc = bacc.Bacc("TRN2", target_bir_lowering=False, debug=False)

    nb = NB_BIAS if use_qk_bias else NB_NOBIAS
    blob_d = nc.dram_tensor("blob", [nb], u8, kind="ExternalInput")
    out_d = nc.dram_tensor("out", [HG, D + 1, S], f32, kind="ExternalOutput")

    blob = blob_d.ap()
    hs_d = blob[OFF_HS : OFF_HS + N_HS].rearrange(
        "(p c s two) -> p c (s two)", c=CC, s=S, two=2
    )
    w_d = blob[OFF_W : OFF_W + N_W].rearrange(
        "(t p c n two) -> t p c (n two)", t=3, p=128, c=CC, n=HGDIM, two=2
    )
    g_d = blob[OFF_G : OFF_G + N_G].rearrange(
        "(hp par a s i) -> hp par a s i", hp=3, par=2, a=9, s=2, i=S
    )

    # j-side bias rows (fp8): [P^ ; pr ; P^] folded -> [9, 2, S]
    pos = np.arange(S, dtype=np.float64) / (S - 1)
    Pp = (np.stack([(pos**b) for b in range(ORDER + 1)], 0) * SB).astype(np.float32)
    Phat, Pres = _bias_rows(Pp)
    pT9 = np.concatenate(
        [_fold35(Phat.view(np.uint8)), _fold35(Pres.view(np.uint8)), _fold35(Phat.view(np.uint8))], 0
    )
    pT_d = nc.inline_tensor(pT9, name="pT9")

    with tile.TileContext(nc) as tc:
        import contextlib

        with contextlib.ExitStack() as ctx:
            consts = ctx.enter_context(tc.tile_pool(name="consts", bufs=1))
            hsT = consts.tile([128, CC, S], bf16, name="hsT")
            w_sb = [consts.tile([128, CC, HGDIM], bf16, name=f"w{t}") for t in range(3)]
            nc.sync.dma_start(out=w_sb[0][:].bitcast(u8), in_=w_d[0])
            nc.sync.dma_start(out=hsT[:, 0, :].bitcast(u8), in_=hs_d[:, 0, :])
            nc.sync.dma_start(out=hsT[:, 1, :].bitcast(u8), in_=hs_d[:, 1, :])
            nc.sync.dma_start(out=w_sb[1][:].bitcast(u8), in_=w_d[1])
            nc.sync.dma_start(out=hsT[:, 2, :].bitcast(u8), in_=hs_d[:, 2, :])
            nc.sync.dma_start(out=w_sb[2][:].bitcast(u8), in_=w_d[2])
            for cc in range(3, CC):
                nc.sync.dma_start(out=hsT[:, cc, :].bitcast(u8), in_=hs_d[:, cc, :])
            v8 = consts.tile([128, JC // 2, HG, VP], fp8, name="v8")
            vlo = consts.tile([128, JC // 2, HG, VP], fp8, name="vlo")
            vb = consts.tile([128, JC // 2, HG, VPB], bf16, name="vb")
            nc.vector.memset(v8[:, :, :, D : D + 1], SV)
            nc.vector.memset(vlo[:, :, :, D : D + 1], 0.0)
            nc.vector.memset(vb[:, :, :, D : D + 1], SV)
            if use_qk_bias:
                bqk_d = blob[OFF_BQK : OFF_BQK + N_BQK].rearrange(
                    "(t hp p four) -> p t hp four", t=2, hp=3, p=128, four=4
                )
                bqk = consts.tile([128, 2, 3], f32)
                nc.sync.dma_start(out=bqk[:].bitcast(u8), in_=bqk_d)

            aug = ctx.enter_context(tc.tile_pool(name="aug", bufs=2))
            psp = ctx.enter_context(tc.tile_pool(name="psp", bufs=4, space="PSUM"))
            expp = ctx.enter_context(tc.tile_pool(name="expp", bufs=6))
            expb = ctx.enter_context(tc.tile_pool(name="expb", bufs=6))
            stage = ctx.enter_context(tc.tile_pool(name="stage", bufs=12))
            ctxs = ctx.enter_context(tc.tile_pool(name="ctxs", bufs=2))

            # ---- V projection (bf16): v' for all 6 heads; fp8 + residual +
            # bf16 copies per chunk parity ----
            def v_thunks():
                ths = []

                def mk(jc):
                    def f():
                        vp = psp.tile([128, 1024], f32, tag="ps", name="vp")
                        for cc in range(CC):
                            nc.tensor.matmul(
                                vp[:, 0:HGDIM],
                                hsT[:, cc, jc * 128 : (jc + 1) * 128],
                                w_sb[2][:, cc, :],
                                start=(cc == 0),
                                stop=(cc == CC - 1),
                            )
                        vph = vp[:, 0:HGDIM].rearrange("p (h d) -> p h d", d=D)
                        if jc % 2 == 0:
                            nc.vector.tensor_copy(vb[:, jc // 2, :, 0:D], vph)
                        else:
                            a = (jc - 1) // 2
                            nc.vector.tensor_copy(v8[:, a, :, 0:D], vph)
                            nc.vector.tensor_sub(
                                out=vlo[:, a, :, 0:D], in0=vph, in1=v8[:, a, :, 0:D]
                            )
                    return f

                for jc in range(JC):
                    ths.append(mk(jc))
                return ths

            aug_tiles = {}

            def make_aug(hp):
                # folded aug [105, 2, S]: parts 0..31 q^/k^, 32..63 q^/kr,
                # 64..95 qr/k^, 96..104 bias rows
                qe = aug.tile([NAUGP, 2, S], fp8, tag="qaug_e")
                qo = aug.tile([NAUGP, 2, S], fp8, tag="qaug_o")
                ke = aug.tile([NAUGP, 2, S], fp8, tag="kaug_e")
                ko = aug.tile([NAUGP, 2, S], fp8, tag="kaug_o")
                nc.gpsimd.dma_start(out=qe[96:105].bitcast(u8), in_=g_d[hp, 0])
                nc.gpsimd.dma_start(out=qo[96:105].bitcast(u8), in_=g_d[hp, 1])
                nc.gpsimd.dma_start(out=ke[96:105].bitcast(u8), in_=pT_d[:])
                nc.gpsimd.dma_start(out=ko[96:105].bitcast(u8), in_=pT_d[:])
                aug_tiles[hp] = (qe, qo, ke, ko)

            def proj_thunks(hp, t, half, act_hat=False):
                """One (tensor, i-half) bf16 projection: 6 cc steps x2 nn into
                a PSUM tile; epilogue: fp8 hat copies (DVE, or ACT at startup),
                fp8 residual subs (DVE), and 6 partition-fold DMAs (split
                across the Pool-SWDGE and SP-HWDGE queues)."""
                qe, qo, ke, ko = aug_tiles[hp]
                dst_e, dst_o = (qe, qo) if t == 0 else (ke, ko)
                st = {}

                def step(cc):
                    def f():
                        if cc == 0:
                            st["pp"] = psp.tile([128, 1024], f32, tag="ps", name="pp")
                        pp = st["pp"]
                        for nn in range(2):
                            nc.tensor.matmul(
                                pp[:, nn * 512 : (nn + 1) * 512],
                                w_sb[t][:, cc, hp * 128 : (hp + 1) * 128],
                                hsT[
                                    :,
                                    cc,
                                    half * 1024 + nn * 512 : half * 1024 + (nn + 1) * 512,
                                ],
                                start=(cc == 0),
                                stop=(cc == CC - 1),
                            )
                    return f

                def fin(par, dst):
                    def f():
                        pp = st["pp"]
                        fs = slice(half * 1024, (half + 1) * 1024)
                        ps = pp[0:D, :] if par == 0 else pp[D:128, :]
                        hat = stage.tile([64, 1024], fp8, tag="stage", name="hat")
                        res = stage.tile([64, 1024], fp8, tag="stage", name="res")
                        if use_qk_bias:
                            nc.scalar.activation(
                                hat[:], ps, Ident, bias=bqk[par * D : (par + 1) * D, t, hp]
                            )
                            nc.vector.memset(res[:], 0.0)
                        elif act_hat:
                            nc.scalar.activation(hat[:], ps, Ident)
                            nc.vector.tensor_sub(out=res[:], in0=ps, in1=hat[:])
                        else:
                            nc.vector.tensor_copy(hat[:], ps)
                            nc.vector.tensor_sub(out=res[:], in0=ps, in1=hat[:])
                        eng3 = nc.scalar if act_hat else nc.gpsimd
                        if t == 0:  # q-side: q^, q^, qr
                            nc.gpsimd.dma_start(out=dst[0:32, :, fs], in_=hat[:])
                            nc.sync.dma_start(out=dst[32:64, :, fs], in_=hat[:])
                            eng3.dma_start(out=dst[64:96, :, fs], in_=res[:])
                        else:  # k-side: k^, kr, k^
                            nc.sync.dma_start(out=dst[0:32, :, fs], in_=hat[:])
                            eng3.dma_start(out=dst[32:64, :, fs], in_=res[:])
                            nc.gpsimd.dma_start(out=dst[64:96, :, fs], in_=hat[:])
                    return f

                return [step(cc) for cc in range(CC)] + [fin(0, dst_e), fin(1, dst_o)]

            def tail_thunks(h, ctx_sb):
                return []

            def attn_section(
                h, ihalf, ctx_sb, fillq, prev_defer, sec_idx=0, last=False
            ):
                """16-jc loop for one (head, i-half).  Even chunks: ACT
                exp->bf16 + bf16 ctx.  Odd chunks: fp8 pair tiles + DoubleRow
                ctx chains (v8 then vlo).  Tail work deferred to next section."""
                hp, par = divmod(h, 2)
                qa = aug_tiles[hp][par]
                ka = aug_tiles[hp][2 + par]
                st = {"started": False}
                ebs = [None] * (JC // 2)
                eps = [None] * (JC // 4)
                dve_jcs = set(DVE_ORDER[: N_DVE[sec_idx]])

                def ctx_mm(lhsT, rhs, last, perf_mode=None):
                    first = not st["started"]
                    st["started"] = True
                    for nn in range(2):
                        rr = (
                            rhs[:, nn * 512 : (nn + 1) * 512]
                            if perf_mode is None
                            else rhs[:, :, nn * 512 : (nn + 1) * 512]
                        )
                        nc.tensor.matmul(
                            st["cp"][0 : D + 1, nn * 512 : (nn + 1) * 512],
                            lhsT,
                            rr,
                            start=first,
                            stop=last,
                            perf_mode=perf_mode,
                        )

                def emit_ctx_b(c, last=False):
                    if "cp" not in st:
                        st["cp"] = psp.tile([128, 1024], f32, tag="ps", name="cp")
                    ctx_mm(vb[:, c // 2, h, 0 : D + 1], ebs[c // 2][:], last)

                def emit_ctx_a(q, last=False):
                    ctx_mm(v8[:, 2 * q : 2 * q + 2, h, 0 : D + 1], eps[q], False, DR)
                    ctx_mm(vlo[:, 2 * q : 2 * q + 2, h, 0 : D + 1], eps[q], last, DR)

                def final_copy():
                    nc.vector.tensor_copy(
                        ctx_sb[:, ihalf * 1024 : (ihalf + 1) * 1024],
                        st["cp"][0 : D + 1, :],
                    )
                    nc.gpsimd.dma_start(
                        out=out_d[h][:, ihalf * 1024 : (ihalf + 1) * 1024],
                        in_=ctx_sb[:, ihalf * 1024 : (ihalf + 1) * 1024],
                    )

                for jc in range(JC):
                    if jc < len(prev_defer):
                        prev_defer[jc]()
                    sp = psp.tile([128, 1024], f32, tag="ps", name="sp")
                    for nn in range(2):
                        nc.tensor.matmul(
                            sp[:, nn * 512 : (nn + 1) * 512],
                            ka[:, :, jc * 128 : (jc + 1) * 128],
                            qa[
                                :,
                                :,
                                ihalf * 1024 + nn * 512 : ihalf * 1024 + (nn + 1) * 512,
                            ],
                            start=True,
                            stop=True,
                            perf_mode=DR,
                        )
                    if jc % 2 == 0:
                        ebs[jc // 2] = expb.tile([128, 1024], bf16, tag="expb", name="eb")
                        nc.scalar.activation(
                            ebs[jc // 2][:], sp[:], Exp, scale=1.0 / SCORE_SCALE
                        )
                    else:
                        a = (jc - 1) // 2
                        p, s = divmod(a, 2)
                        if s == 0:
                            eps[p] = expp.tile([128, 2, 1024], fp8, tag="expp", name="ep")
                        if jc in dve_jcs:
                            nc.vector.tensor_scalar(
                                eps[p][:, s, :].bitcast(u8),
                                sp[:],
                                SCH_C1,
                                SCH_C2,
                                MUL,
                                ADD,
                            )
                        else:
                            nc.scalar.activation(
                                eps[p][:, s, :], sp[:], Exp, scale=1.0 / SCORE_SCALE
                            )
                    if jc >= 2:
                        for _ in range(2):
                            if fillq:
                                fillq.pop(0)()
                    # lagged ctx: even chunk c at jc=c+2; fp8 pair q at jc=4q+5
                    if jc >= 2 and jc % 2 == 0:
                        emit_ctx_b(jc - 2)
                    elif jc >= 5 and (jc - 1) % 4 == 0:
                        emit_ctx_a((jc - 5) // 4)
                for th in fillq:
                    th()
                ret = [
                    (lambda: emit_ctx_b(JC - 2)),
                    (lambda: emit_ctx_a(JC // 4 - 1, last=True)),
                    final_copy,
                ]
                if last:
                    for th in ret:
                        th()
                    ret = []
                return ret

            # ---- schedule ----
            make_aug(0)
            vths = v_thunks()
            pre = (
                proj_thunks(0, 0, 0, act_hat=True)
                + proj_thunks(0, 1, 0, act_hat=True)
                + [vths[jc] for jc in (0, 1, 2, 3, 4)]
            )
            for th in pre:
                th()

            ctx_sbs = {}
            defer = []
            pending_tail = []
            # fill order respects emission-order deadlines (a PE consumer must
            # never precede its producer in the PE stream)
            carry = (
                proj_thunks(0, 1, 1)
                + [vths[jc] for jc in range(5, JC)]
                + proj_thunks(0, 0, 1)
            )
            for hp in range(3):
                if hp + 1 < 3:
                    make_aug(hp + 1)
                    pj = []
                    for t in range(2):
                        for half in range(2):
                            pj += proj_thunks(hp + 1, t, half)
                else:
                    pj = []
                h_e, h_o = 2 * hp, 2 * hp + 1
                ctx_sbs[h_e] = ctxs.tile([D + 1, S], f32, tag="ctx_sb", name="ctx_sb")
                ctx_sbs[h_o] = ctxs.tile([D + 1, S], f32, tag="ctx_sb", name="ctx_sb")

                nfill = 28
                if hp == 0:
                    s0_fill = carry + pending_tail
                    s1_fill = pj[:nfill]
                    pj = pj[nfill:]
                else:
                    q0 = carry + pending_tail
                    s0_fill, rest = q0[:nfill], q0[nfill:]
                    s1_fill = rest + pj[: nfill - len(rest)]
                    pj = pj[nfill - len(rest) :]
                carry = []

                defer = attn_section(
                    h_e, 0, ctx_sbs[h_e], s0_fill, defer, sec_idx=4 * hp
                )
                defer = attn_section(
                    h_e, 1, ctx_sbs[h_e], s1_fill, defer, sec_idx=4 * hp + 1
                )

                te = tail_thunks(h_e, ctx_sbs[h_e])
                s2_fill, pj = (te + pj)[:nfill], (te + pj)[nfill:]
                s3_fill = pj
                defer = attn_section(
                    h_o, 0, ctx_sbs[h_o], s2_fill, defer, sec_idx=4 * hp + 2
                )
                defer = attn_section(
                    h_o,
                    1,
                    ctx_sbs[h_o],
                    s3_fill,
                    defer,
                    sec_idx=4 * hp + 3,
                    last=(hp == 2),
                )

                pending_tail = tail_thunks(h_o, ctx_sbs[h_o])

            for th in defer:
                th()
            for th in pending_tail:
                th()

    nc.finalize()
    return nc


def _get_nc(use_qk_bias: bool):
    key = ("prog", use_qk_bias)
    if key not in _CACHE:
        _CACHE[key] = _build_program(use_qk_bias)
    return _CACHE[key]


def _get_runner(use_qk_bias: bool):
    """Cached PJRT runner: one jitted executable + resident zero-output
    buffers; per call only the input blob is uploaded."""
    key = ("runner", use_qk_bias)
    if key in _CACHE:
        return _CACHE[key]

    nc = _get_nc(use_qk_bias)

    import jax
    from jax.sharding import Mesh, PartitionSpec
    from jax.experimental.shard_map import shard_map
    import concourse.mybir as mybir
    from concourse.bass2jax import (
        _bass_exec_p,
        install_neuronx_cc_hook,
        partition_id_tensor,
    )

    install_neuronx_cc_hook()
    partition_name = nc.partition_id_tensor.name if nc.partition_id_tensor else None
    in_names, out_names, out_avals, zero_outs = [], [], [], []
    for alloc in nc.m.functions[0].allocations:
        if not isinstance(alloc, mybir.MemoryLocationSet):
            continue
        name = alloc.memorylocations[0].name
        if alloc.kind == "ExternalInput":
            if name != partition_name:
                in_names.append(name)
        elif alloc.kind == "ExternalOutput":
            out_names.append(name)
            shape = tuple(alloc.tensor_shape)
            dtype = mybir.dt.np(alloc.dtype)
            out_avals.append(jax.core.ShapedArray(shape, dtype))
            zero_outs.append(np.zeros(shape, dtype))
    assert in_names == ["blob"] and out_names == ["out"]
    all_in_names = list(in_names) + list(out_names)
    if partition_name is not None:
        all_in_names.append(partition_name)

    def _body(*args):
        operands = list(args)
        if partition_name is not None:
            operands.append(partition_id_tensor())
        return tuple(
            _bass_exec_p.bind(
                *operands,
                out_avals=tuple(out_avals),
                in_names=tuple(all_in_names),
                out_names=tuple(out_names),
                lowering_input_output_aliases=(),
                sim_require_finite=True,
                sim_require_nnan=True,
                nc=nc,
            )
        )

    devices = jax.devices()[:NCORES]
    mesh = Mesh(np.asarray(devices), ("core",))
    sharded = jax.jit(
        shard_map(
            _body,
            mesh=mesh,
            in_specs=(PartitionSpec("core"),) * 2,
            out_specs=(PartitionSpec("core"),),
            check_rep=False,
        ),
        keep_unused=True,
    )
    dev_zeros = jax.device_put(
        np.zeros(
            (NCORES * zero_outs[0].shape[0], *zero_outs[0].shape[1:]),
            zero_outs[0].dtype,
        )
    )
    out_shape = tuple(out_avals[0].shape)

    def run(blobs: np.ndarray):
        (out,) = sharded(blobs, dev_zeros)
        return np.asarray(out).reshape(NCORES, *out_shape)

    _CACHE[key] = run
    return run


def _make_in_maps(hidden_states, Wq, bq, Wk, bk, Wv, bv, alphas, use_qk_bias):
    """Per-core packed blobs (uint8 1-D), as list of dicts keyed 'blob'."""
    import ml_dtypes

    bf = ml_dtypes.bfloat16
    G, _ = _cheb_factors(alphas)  # [12, 6, S] f32

    hs_b = []
    for b in range(B):
        t = np.ascontiguousarray(hidden_states[b].T)      # [768, S]
        t = t.reshape(CC, 128, S).transpose(1, 0, 2)      # [128, CC, S]
        hs_b.append(np.ascontiguousarray(t).astype(bf).view(np.uint8).ravel())

    w_hg, g_hg, bqk_hg = [], [], []
    for hg in range(2):
        rows = slice(hg * HGDIM, (hg + 1) * HGDIM)
        ws = []
        for W, s in ((Wq, SQ), (Wk, SK), (Wv, SV)):
            wt = np.ascontiguousarray(W[rows, :].T) * s       # [768, 384]
            wt = wt.reshape(CC, 128, HGDIM).transpose(1, 0, 2)
            ws.append(np.ascontiguousarray(wt).astype(bf).view(np.uint8).ravel())
        w_hg.append(np.concatenate(ws))
        gg = np.empty((3, 2, 9, 2, S), dtype=np.uint8)
        for hp in range(3):
            for par in range(2):
                Ghat, Gres = _bias_rows(SB * G[hg * HG + 2 * hp + par])
                gg[hp, par] = np.concatenate(
                    [
                        _fold35(Ghat.view(np.uint8)),
                        _fold35(Ghat.view(np.uint8)),
                        _fold35(Gres.view(np.uint8)),
                    ],
                    0,
                )
        g_hg.append(gg.ravel())
        bqk_hg.append(
            np.ascontiguousarray(
                np.stack([bq[rows] * SQ, bk[rows] * SK], 0), dtype=np.float32
            )
            .ravel()
            .view(np.uint8)
        )

    in_maps = []
    for core in range(NCORES):
        b, hg = divmod(core, 2)
        parts = [hs_b[b], w_hg[hg], g_hg[hg]]
        if use_qk_bias:
            parts.append(bqk_hg[hg])
        in_maps.append({"blob": np.concatenate(parts)})
    return in_maps


def kernel(hidden_states, Wq, bq, Wk, bk, Wv, bv, alphas):
    hidden_states = np.asarray(hidden_states, dtype=np.float32)
    Wq = np.asarray(Wq, dtype=np.float32)
    Wk = np.asarray(Wk, dtype=np.float32)
    Wv = np.asarray(Wv, dtype=np.float32)
    bq = np.asarray(bq, dtype=np.float32)
    bk = np.asarray(bk, dtype=np.float32)
    bv = np.asarray(bv, dtype=np.float32)
    alphas = np.asarray(alphas, dtype=np.float32)

    use_qk_bias = bool(np.any(bq) or np.any(bk))
    run = _get_runner(use_qk_bias)
    in_maps = _make_in_maps(
        hidden_states, Wq, bq, Wk, bk, Wv, bv, alphas, use_qk_bias
    )
    blobs = np.concatenate([m["blob"] for m in in_maps])
    res = run(blobs)  # [8, HG, 65, S] f32

    resf = np.asarray(res, dtype=np.float32)
    num = resf[:, :, 0:D, :]
    den = resf[:, :, D : D + 1, :]
    ctx = num / den
    ctx = ctx.transpose(0, 3, 1, 2).reshape(NCORES, S, HGDIM)

    out = np.empty((B, S, HIDDEN), dtype=np.float32)
    add_bv = bool(np.any(bv))
    for core in range(NCORES):
        b, hg = divmod(core, 2)
        o = ctx[core]
        if add_bv:
            o = o + bv[hg * HGDIM : (hg + 1) * HGDIM][None, :]
        out[b, :, hg * HGDIM : (hg + 1) * HGDIM] = o
    return out
